# revision 4
# baseline (speedup 1.0000x reference)
# Trainium2 Bass kernel v2 for nn_DiT_89086211653924 (windowed-attention video
# DiT).  Same sharding as v1: core c -> (batch c//4, temporal group c%4), 1024
# tokens per core, zero collectives.
#
# v2 changes vs v1:
#  - all matmuls in bf16 (4x PE rate), activations stored bf16, psum fp32
#  - LayerNorm stats via gpsimd partition_all_reduce (13 ops vs ~30)
#  - attention softmax batched per window group: one exp per 4 heads (multi-
#    bank PSUM AP), denominators via ones-matmuls, reciprocal rows broadcast
#    via gpsimd partition_broadcast
#  - block-g (4x4x4 window) scores computed per window-PAIR with a +/-25
#    penalty rank-1 term so cross-window attention is suppressed exactly by
#    the softmax (saves half the score/ob matmuls)
#  - K-projection bias dropped (softmax-invariant), V-projection bias folded
#    into the out-projection bias on the host
#  - weights shipped as 4 packed tensors (2 per block), few big DMAs
#  - AOT-cached jit executable + on-device input caching so repeated calls
#    do not pay client-side recompile/transfer
import sys

sys.path.insert(0, "/opt/trn_rl_repo")

import numpy as np
import ml_dtypes

import concourse.bacc as bacc
import concourse.mybir as mybir
import concourse.tile as tile
import concourse.bass as bass
from concourse.bass_utils import run_bass_kernel_spmd

F32 = mybir.dt.float32
BF16 = mybir.dt.bfloat16
AF = mybir.ActivationFunctionType
ALU = mybir.AluOpType
ReduceOp = bass.bass_isa.ReduceOp
BFNP = ml_dtypes.bfloat16

B, CIN, F, IMG, P = 2, 3, 16, 64, 4
PD = IMG // P          # 16 patches per side
NP = PD * PD           # 256 patches per frame
C, NH = 512, 8
HD = C // NH           # 64
S = 16                 # action context length
SCALE = float(1.0 / np.sqrt(HD))
M = 1024               # tokens per core (4 frames)
CT = 4                 # c tiles of 128
EPS = 1e-5
PEN = 5.0              # sqrt of window penalty (exactly representable in bf16)

# weight column offsets inside each half-pack (units of C=512 columns)
# half A: wq1 wk1 wv1 wo1 wq2   half B: wk2 wv2 wo2 wf1 wf2 [wh]
OA = {"wq1": 0, "wk1": 512, "wv1": 1024, "wo1": 1536, "wq2": 2048}
OB = {"wk2": 0, "wv2": 512, "wo2": 1024, "wf1": 1536, "wf2": 2048, "wh": 2560}
NWA, NWB = 2560, 2608

_BUILD_CACHE = {}

import os as _os
_STAGE = int(_os.environ.get("DIT_STAGE", "99"))


# ---------------------------------------------------------------- host prep

def _w4(a):
    """[512, N] k-major weight -> [4, 128, N] bf16."""
    return np.ascontiguousarray(a.reshape(4, 128, -1)).astype(BFNP)


def _colscal(a):
    """[512] bias -> [128, 4] per-partition fp32 column layout."""
    return np.ascontiguousarray(a.reshape(4, 128).T.astype(np.float32))


def _mprime_index():
    a, bb, f, i, j = np.meshgrid(
        np.arange(4), np.arange(4), np.arange(4), np.arange(4), np.arange(4),
        indexing="ij")
    return (f * 256 + (4 * a + i) * 16 + 4 * bb + j).reshape(-1)


def _fold_block(p, w):
    g, b = w[p + "_ln_g"], w[p + "_ln_b"]
    qkv1, wo1, bo1 = w[p + "_qkv1"], w[p + "_wo1"], w[p + "_bo1"]
    qkv2, wo2, bo2 = w[p + "_qkv2"], w[p + "_wo2"], w[p + "_bo2"]
    f1w, f1b = w[p + "_fc1w"], w[p + "_fc1b"]
    f2w, f2b = w[p + "_fc2w"], w[p + "_fc2b"]
    d = {}
    d["wq1"] = (g[0][:, None] * qkv1[0]) * SCALE
    d["bq1"] = (b[0] @ qkv1[0]) * SCALE
    d["wk1"] = g[0][:, None] * qkv1[1]          # k bias dropped (softmax-inv)
    d["wv1"] = g[0][:, None] * qkv1[2]
    bv1 = b[0] @ qkv1[2]
    d["wo1"] = wo1
    d["bo1"] = bo1 + bv1 @ wo1                  # v bias folded through softmax
    d["wq2"] = (g[1][:, None] * qkv2[0]) * SCALE
    d["bq2"] = (b[1] @ qkv2[0]) * SCALE
    d["wk2"] = qkv2[1]
    d["wv2"] = qkv2[2]
    d["wo2"], d["bo2"] = wo2, bo2
    d["wf1"] = g[2][:, None] * f1w
    d["bf1"] = b[2] @ f1w + f1b
    d["wf2"], d["bf2"] = f2w, f2b
    return d


def host_prep(inputs):
    """Full inputs -> list of 8 per-core input maps (shared weights aliased)."""
    w = {k: np.asarray(v) for k, v in inputs.items()}
    x = w["x"].astype(np.float32)
    actions = np.asarray(w["actions"])

    shared = {}
    shared["wp"] = np.ascontiguousarray(
        w["conv_w"].reshape(C, CIN * P * P).T).astype(BFNP)       # [48, 512]
    shared["bh"] = np.ascontiguousarray(
        w["head_b"].reshape(48, 1).astype(np.float32))

    for p in ("s", "g"):
        fb = _fold_block(p, w)
        wa = np.concatenate(
            [_w4(fb[n]) for n in ("wq1", "wk1", "wv1", "wo1", "wq2")], axis=2)
        wbl = [_w4(fb[n]) for n in ("wk2", "wv2", "wo2", "wf1", "wf2")]
        if p == "g":
            wbl.append(_w4(w["head_w"]))                           # [4,128,48]
        else:
            wbl.append(np.zeros((4, 128, 48), BFNP))
        wb = np.concatenate(wbl, axis=2)
        shared[p + "wa"] = np.ascontiguousarray(wa)
        shared[p + "wb"] = np.ascontiguousarray(wb)
        shared[p + "bias"] = np.ascontiguousarray(np.concatenate(
            [_colscal(fb[n]) for n in ("bq1", "bo1", "bq2", "bo2",
                                       "bf1", "bf2")], axis=1))    # [128, 24]

    # cross-attn selector matrices: R[p, ct, q] = rec[32*((2ct+(p>=64))%4), q]
    sel = np.zeros((4, 128, 128), np.float32)
    for ct in range(4):
        for pp in range(128):
            h = 2 * ct + (1 if pp >= 64 else 0)
            sel[ct, 32 * (h % 4), pp] = 1.0
    shared["sel"] = sel.astype(BFNP)
    same = (np.arange(128)[:, None] < 64) == (np.arange(128)[None, :] < 64)
    shared["wmask"] = np.ascontiguousarray(
        np.tile(same.astype(np.float32), (1, 8))).astype(BFNP)

    # positional embedding folded with conv bias: pe[c, (f, ph, pw)]
    pe_all = (w["conv_b"][None, None, :] + w["t_pos"][:, None, :]
              + w["sp_pos"][None, :, :])                           # [F, NP, C]

    per_core = []
    for c in range(8):
        b, t = c // 4, c % 4
        m = dict(shared)
        xs = x[b, :, 4 * t:4 * t + 4].reshape(CIN, 4, PD, P, PD, P)
        xs = xs.transpose(0, 3, 5, 1, 2, 4)        # (c, p, q, f, ph, pw)
        m["xpt"] = np.ascontiguousarray(
            xs.reshape(CIN * P * P, M)).astype(BFNP)
        pe = pe_all[4 * t:4 * t + 4].reshape(M, C).T               # [C, M]
        m["pe"] = np.ascontiguousarray(pe.reshape(4, 128, M)).astype(BFNP)
        ctx = (w["act_table"][actions[b]] + w["act_pos"][0])
        m["ctxt"] = np.ascontiguousarray(
            ctx.T.reshape(4, 128, S)).astype(BFNP)
        per_core.append(m)
    return per_core


# ---------------------------------------------------------------- device

class Dev:
    pass


class Pools:
    pass


def _declare_inputs(nc):
    d = Dev()
    d.nc = nc
    mkb = lambda name, shape: nc.dram_tensor(name, shape, BF16,
                                             kind="ExternalInput")
    mkf = lambda name, shape: nc.dram_tensor(name, shape, F32,
                                             kind="ExternalInput")
    d.xpt = mkb("xpt", [48, M])
    d.pe = mkb("pe", [4, 128, M])
    d.ctxt = mkb("ctxt", [4, 128, S])
    d.wp = mkb("wp", [48, C])
    d.bh = mkf("bh", [48, 1])
    d.sel = mkb("sel", [4, 128, 128])
    d.wmask = mkb("wmask", [128, 1024])
    for p in ("s", "g"):
        setattr(d, p + "wa", mkb(p + "wa", [4, 128, NWA]))
        setattr(d, p + "wb", mkb(p + "wb", [4, 128, NWB]))
        setattr(d, p + "bias", mkf(p + "bias", [128, 24]))
    d.out = nc.dram_tensor("out", [48, M], F32, kind="ExternalOutput")
    return d


def _dbg_out(d, p, t_sb):
    nc = d.nc
    outT = p.cpool.tile([48, M], F32, name="outT", tag="outT")
    nc.vector.tensor_copy(out=outT, in_=t_sb)
    nc.sync.dma_start(out=d.out.ap(), in_=outT)


def _load_whalf(d, p, dram, name):
    nc = d.nc
    n = dram.shape[2]
    t = p.wpool.tile([128, 4, n], BF16, name=name, tag="whalf")
    nc.sync.dma_start(out=t, in_=dram.ap().transpose([1, 0, 2]))
    return t


def _linear_fm(d, p, w_sb, col0, x_sb, out_sb, mode, bias_col=None):
    """Feature-major linear over tokens.  mode: 'act' (ACT identity+bias),
    'copy' (DVE copy, no bias), 'res' (h += ps + bias), 'gelu'."""
    nc = d.nc
    for ct in range(CT):
        for ms in range(2):
            ps = p.ps.tile([128, 512], F32, name="lin_ps", tag="ps")
            for kt in range(CT):
                nc.tensor.matmul(
                    ps, w_sb[:, kt, col0 + ct * 128:col0 + (ct + 1) * 128],
                    x_sb[:, kt, ms * 512:(ms + 1) * 512],
                    start=(kt == 0), stop=(kt == 3))
            sl = out_sb[:, ct, ms * 512:(ms + 1) * 512]
            if mode == "res":
                nc.vector.scalar_tensor_tensor(
                    out=sl, in0=ps, scalar=bias_col[:, ct:ct + 1], in1=sl,
                    op0=ALU.add, op1=ALU.add)
            elif mode == "copy":
                nc.vector.tensor_copy(out=sl, in_=ps)
            elif mode == "act":
                nc.scalar.activation(out=sl, in_=ps, func=AF.Identity,
                                     bias=bias_col[:, ct:ct + 1], scale=1.0)
            elif mode == "gelu":
                nc.scalar.activation(out=sl, in_=ps, func=AF.Gelu,
                                     bias=bias_col[:, ct:ct + 1], scale=1.0)


def _v_proj(d, p, w_sb, col0, x_sb, v_sb):
    """Token-major value projection v_sb[128 tok, mt, C] (no bias)."""
    nc = d.nc
    for mt in range(8):
        ps = p.ps.tile([128, 512], F32, name="v_ps", tag="ps")
        for kt in range(CT):
            nc.tensor.matmul(
                ps, x_sb[:, kt, mt * 128:(mt + 1) * 128],
                w_sb[:, kt, col0:col0 + 512],
                start=(kt == 0), stop=(kt == 3))
        nc.scalar.copy(out=v_sb[:, mt, :], in_=ps)


def _ln(d, p, h_sb, z_sb):
    """z = (h - mean_c) * rsqrt(var_c + eps) via gpsimd partition allreduce."""
    nc = d.nc
    sq = p.state.tile([128, CT, M], BF16, name="ln_sq", tag="scratch")
    nc.scalar.activation(out=sq, in_=h_sb, func=AF.Square)
    c2 = p.lncomb.tile([128, 2, M], BF16, name="ln_c2", tag="lncomb")
    nc.vector.tensor_add(c2, h_sb[:, 0:2, :], h_sb[:, 2:4, :])
    cs = p.lncs.tile([128, M], BF16, name="ln_cs", tag="lncs")
    nc.vector.tensor_add(cs, c2[:, 0, :], c2[:, 1, :])
    d2 = p.lncomb.tile([128, 2, M], BF16, name="ln_d2", tag="lncomb")
    nc.vector.tensor_add(d2, sq[:, 0:2, :], sq[:, 2:4, :])
    ds_ = p.lncs.tile([128, M], BF16, name="ln_ds", tag="lncs")
    nc.vector.tensor_add(ds_, d2[:, 0, :], d2[:, 1, :])
    sh = p.lnar.tile([128, M], BF16, name="ln_sh", tag="lnar")
    nc.gpsimd.partition_all_reduce(sh, cs, 128, ReduceOp.add)
    sqs = p.lnar.tile([128, M], BF16, name="ln_sqs", tag="lnar")
    nc.gpsimd.partition_all_reduce(sqs, ds_, 128, ReduceOp.add)
    mu2 = p.lnt.tile([128, M], BF16, name="ln_mu2", tag="lnt")
    nc.vector.tensor_mul(mu2, sh, sh)
    varu = p.lnt.tile([128, M], BF16, name="ln_varu", tag="lnt")
    # varu = mu2/512 - sumsq  (= -512*var)
    nc.vector.scalar_tensor_tensor(out=varu, in0=mu2, scalar=1.0 / C,
                                   in1=sqs, op0=ALU.mult, op1=ALU.subtract)
    sd = p.lnt.tile([128, M], BF16, name="ln_sd", tag="lnt")
    nc.scalar.activation(out=sd, in_=varu, func=AF.Sqrt,
                         bias=d.epscol, scale=-1.0 / C)
    r = p.lnt.tile([128, M], BF16, name="ln_r", tag="lnt")
    with nc.allow_low_precision(reason="rel tolerance 2e-2"):
        nc.vector.reciprocal(r, sd)
    psi = p.lnt.tile([128, M], BF16, name="ln_psi", tag="lnt")
    nc.vector.scalar_tensor_tensor(out=psi, in0=sh, scalar=-1.0 / C,
                                   in1=r, op0=ALU.mult, op1=ALU.mult)
    rb = r.unsqueeze(1).broadcast_to([128, CT, M])
    pb = psi.unsqueeze(1).broadcast_to([128, CT, M])
    nc.vector.tensor_mul(z_sb, h_sb, rb)
    nc.vector.tensor_add(z_sb, z_sb, pb)


def _attn_s(d, p, qt, kt, v_sb, attn):
    """Block-s self attention: 4 windows (frames) x 256 tokens, 8 heads."""
    nc = d.nc
    for w in range(4):
        exs = []
        for g in range(2):
            scg = p.psA.tile([128, 4, 512], F32, name="scg_s", tag="psA")
            for hh in range(4):
                h_ = 4 * g + hh
                hp, ct = 64 * (h_ % 2), h_ // 2
                for st in range(2):
                    nc.tensor.matmul(
                        scg[:, hh, st * 256:(st + 1) * 256],
                        kt[hp:hp + 64, ct,
                           w * 256 + st * 128:w * 256 + st * 128 + 128],
                        qt[hp:hp + 64, ct, w * 256:(w + 1) * 256],
                        start=True, stop=True, tile_position=(hp, 0))
            ex = p.ex.tile([128, 4, 2, 256], BF16, name="ex_s", tag="ex")
            nc.scalar.activation(out=ex, in_=scg, func=AF.Exp)
            exs.append(ex)
        dn = p.psA.tile([1, 2048], F32, name="dn_s", tag="psA")
        for h_ in range(8):
            g, hh = h_ // 4, h_ % 4
            for st in range(2):
                nc.tensor.matmul(
                    dn[0:1, h_ * 256:(h_ + 1) * 256],
                    d.ones_col, exs[g][:, hh, st, :],
                    start=(st == 0), stop=(st == 1))
        rrow = p.rrow.tile([1, 2048], BF16, name="rr_s", tag="rrow")
        with nc.allow_low_precision(reason="rel tolerance 2e-2"):
            nc.vector.reciprocal(rrow, dn)
        rb = p.rb.tile([128, 2048], BF16, name="rb_s", tag="rb")
        nc.gpsimd.partition_broadcast(rb, rrow, 128)
        rbv = rb.rearrange("p (h q) -> p h q", h=8)
        for g in range(2):
            in1 = rbv[:, 4 * g:4 * g + 4, :].unsqueeze(2).broadcast_to(
                [128, 4, 2, 256])
            nc.vector.tensor_mul(exs[g], exs[g], in1)
        ob = p.psB.tile([128, 4, 256], F32, name="ob_s", tag="psB")
        for h_ in range(8):
            g, hh = h_ // 4, h_ % 4
            hp, ct = 64 * (h_ % 2), h_ // 2
            for st in range(2):
                nc.tensor.matmul(
                    ob[hp:hp + 64, ct, :],
                    v_sb[:, 2 * w + st, h_ * 64:h_ * 64 + 64],
                    exs[g][:, hh, st, :],
                    start=(st == 0), stop=(st == 1), tile_position=(0, hp))
        nc.scalar.copy(out=attn[:, :, w * 256:(w + 1) * 256], in_=ob)


def _attn_g(d, p, qt, kt, v_sb, attn):
    """Block-g self attention: 16 windows x 64 tokens processed as 8 window
    pairs; cross-window terms masked to exactly 0 after exp.

    Scores are banked by head parity (slice idx = 4*(h%2) + h//2) so all
    concurrent matmuls in a PSUM bank share one contraction row group."""
    nc = d.nc
    for wp in range(8):
        scg = p.psB.tile([128, 8, 128], F32, name="scg_g", tag="psB")
        for h_ in range(8):
            hp, ct = 64 * (h_ % 2), h_ // 2
            idx = 4 * (h_ % 2) + ct
            nc.tensor.matmul(
                scg[:, idx, :],
                kt[hp:hp + 64, ct, wp * 128:(wp + 1) * 128],
                qt[hp:hp + 64, ct, wp * 128:(wp + 1) * 128],
                start=True, stop=True, tile_position=(hp, 0))
        ex = p.ex.tile([128, 8, 128], BF16, name="ex_g", tag="ex")
        nc.scalar.activation(out=ex, in_=scg, func=AF.Exp)
        # zero the cross-window quadrants (exact block-diagonal attention)
        nc.vector.tensor_mul(
            ex, ex, d.wmask_sb.rearrange("p (h q) -> p h q", h=8))
        # psA is idle during block-g self-attention; using it for the
        # denominators lets the next window pair's scores start in psB
        # while this pair's normalization is still in flight.
        dn = p.psA.tile([1, 1024], F32, name="dn_g", tag="psA")
        for half in range(2):
            nc.tensor.matmul(
                dn[0:1, half * 512:(half + 1) * 512],
                d.ones_col, ex[:, 4 * half:4 * half + 4, :].rearrange(
                    "p a b -> p (a b)"),
                start=True, stop=True)
        rrow = p.rrow.tile([1, 1024], BF16, name="rr_g", tag="rrow")
        with nc.allow_low_precision(reason="rel tolerance 2e-2"):
            nc.vector.reciprocal(rrow, dn)
        rb = p.rb.tile([128, 1024], BF16, name="rb_g", tag="rb")
        nc.gpsimd.partition_broadcast(rb, rrow, 128)
        nc.vector.tensor_mul(ex, ex, rb.rearrange("p (h q) -> p h q", h=8))
        ob = p.ps.tile([128, 4, 128], F32, name="ob_g", tag="ps")
        for h_ in range(8):
            hp, ct = 64 * (h_ % 2), h_ // 2
            idx = 4 * (h_ % 2) + ct
            nc.tensor.matmul(
                ob[hp:hp + 64, ct, :],
                v_sb[:, wp, h_ * 64:h_ * 64 + 64],
                ex[:, idx, :],
                start=True, stop=True, tile_position=(0, hp))
        nc.scalar.copy(out=attn[:, :, wp * 128:(wp + 1) * 128], in_=ob)


def _cross_attn(d, p, prefix, wb_sb, qt2, attn):
    """Cross attention to the 16 action-context tokens."""
    nc = d.nc
    # ctx K feature-major [128, 4, 16]
    ktc = p.ktc.tile([128, CT, S], BF16, name=prefix + "ktc", tag="ktc")
    for ct in range(CT):
        ps = p.ps.tile([128, 512], F32, name="ktc_ps", tag="ps")
        for kt in range(CT):
            nc.tensor.matmul(
                ps[:, 0:S],
                wb_sb[:, kt, OB["wk2"] + ct * 128:OB["wk2"] + (ct + 1) * 128],
                d.ctxt_sb[:, kt, :], start=(kt == 0), stop=(kt == 3))
        nc.scalar.copy(out=ktc[:, ct, :], in_=ps[:, 0:S])
    # ctx V token-major [16, 512], replicated to the four 32-strips
    vc = p.vc.tile([128, C], BF16, name=prefix + "vc", tag="vc")
    ps = p.ps.tile([128, 512], F32, name="vc_ps", tag="ps")
    for kt in range(CT):
        nc.tensor.matmul(ps[0:S, :], d.ctxt_sb[:, kt, :],
                         wb_sb[:, kt, OB["wv2"]:OB["wv2"] + 512],
                         start=(kt == 0), stop=(kt == 3))
    nc.scalar.copy(out=vc[0:S, :], in_=ps[0:S, :])
    for q in range(1, 4):
        nc.sync.dma_start(out=vc[32 * q:32 * q + S, :], in_=vc[0:S, :])

    for ms in range(2):
        msl = slice(ms * 512, (ms + 1) * 512)
        sc = p.psB.tile([128, 2, 512], F32, name="sc_x", tag="psB")
        for h_ in range(8):
            g, hh = h_ // 4, h_ % 4
            hp, ct = 64 * (h_ % 2), h_ // 2
            nc.tensor.matmul(
                sc[32 * hh:32 * hh + S, g, :],
                ktc[hp:hp + 64, ct, :],
                qt2[hp:hp + 64, ct, msl],
                start=True, stop=True, tile_position=(hp, 32 * hh))
        exc = p.ex.tile([128, 2, 512], BF16, name="ex_x", tag="ex")
        nc.scalar.activation(out=exc, in_=sc, func=AF.Exp)
        dn = p.psB.tile([128, 2, 512], F32, name="dn_x", tag="psB")
        nc.vector.memset(dn, 1.0)
        for h_ in range(8):
            g, hh = h_ // 4, h_ % 4
            nc.tensor.matmul(
                dn[32 * hh:32 * hh + 1, g, :],
                d.ones_col[32 * hh:32 * hh + S, 0:1],
                exc[32 * hh:32 * hh + S, g, :],
                start=True, stop=True, skip_group_check=True,
                tile_position=(32 * hh, 32 * hh))
        rec = p.rec.tile([128, 2, 512], BF16, name="rec_x", tag="rec")
        with nc.allow_low_precision(reason="rel tolerance 2e-2"):
            nc.vector.reciprocal(rec, dn)
        R = p.psA.tile([128, 4, 512], F32, name="R_x", tag="psA")
        for ct in range(CT):
            nc.tensor.matmul(R[:, ct, :], d.sel_sb[:, ct, :],
                             rec[:, ct // 2, :], start=True, stop=True)
        rsb = p.rsb.tile([128, 4, 512], BF16, name="rsb_x", tag="rsb")
        nc.vector.tensor_copy(out=rsb, in_=R)
        obp = p.psA.tile([128, 4, 512], F32, name="ob_x", tag="psA")
        for h_ in range(8):
            g, hh = h_ // 4, h_ % 4
            hp, ct = 64 * (h_ % 2), h_ // 2
            nc.tensor.matmul(
                obp[hp:hp + 64, ct, :],
                vc[32 * hh:32 * hh + S, h_ * 64:h_ * 64 + 64],
                exc[32 * hh:32 * hh + S, g, :],
                start=True, stop=True, tile_position=(32 * hh, hp))
        nc.vector.tensor_mul(attn[:, :, msl], obp, rsb)


def _block(d, p, prefix, h_sb, dbg_base):
    nc = d.nc
    wa = _load_whalf(d, p, getattr(d, prefix + "wa"), prefix + "wa")
    z = p.state.tile([128, CT, M], BF16, name="z", tag="zbuf")
    attn = p.state.tile([128, CT, M], BF16, name="attn", tag="attnbuf")
    qt = p.state.tile([128, CT, M], BF16, name="qt", tag="qbuf")
    kt = p.state.tile([128, CT, M], BF16, name="kt", tag="kbuf")
    bias = getattr(d, prefix + "bias_sb")

    _ln(d, p, h_sb, z)
    if _STAGE == dbg_base + 0:
        return _dbg_out(d, p, z[0:48, 0, :])
    _linear_fm(d, p, wa, OA["wq1"], z, qt, "act", bias[:, 0:4])
    _linear_fm(d, p, wa, OA["wk1"], z, kt, "copy")
    v_sb = p.state.tile([128, 8, C], BF16, name="v", tag="vbuf")
    _v_proj(d, p, wa, OA["wv1"], z, v_sb)
    if _STAGE == dbg_base + 1:
        return _dbg_out(d, p, qt[0:48, 0, :])
    if prefix == "s":
        _attn_s(d, p, qt, kt, v_sb, attn)
    else:
        _attn_g(d, p, qt, kt, v_sb, attn)
    if _STAGE == dbg_base + 2:
        return _dbg_out(d, p, attn[0:48, 0, :])
    _linear_fm(d, p, wa, OA["wo1"], attn, h_sb, "res", bias[:, 4:8])

    if _STAGE == dbg_base + 3:
        return _dbg_out(d, p, h_sb[0:48, 0, :])
    _ln(d, p, h_sb, z)
    qt2 = p.state.tile([128, CT, M], BF16, name="qt2", tag="kbuf")
    _linear_fm(d, p, wa, OA["wq2"], z, qt2, "act", bias[:, 8:12])
    wb = _load_whalf(d, p, getattr(d, prefix + "wb"), prefix + "wb")
    _cross_attn(d, p, prefix, wb, qt2, attn)
    if _STAGE == dbg_base + 4:
        return _dbg_out(d, p, attn[0:48, 0, :])
    _linear_fm(d, p, wb, OB["wo2"], attn, h_sb, "res", bias[:, 12:16])
    if _STAGE == dbg_base + 5:
        return _dbg_out(d, p, h_sb[0:48, 0, :])

    _ln(d, p, h_sb, z)
    gbuf = p.state.tile([128, CT, M], BF16, name="gbuf", tag="scratch")
    _linear_fm(d, p, wb, OB["wf1"], z, gbuf, "gelu", bias[:, 16:20])
    _linear_fm(d, p, wb, OB["wf2"], gbuf, h_sb, "res", bias[:, 20:24])
    return wb


def _body(d, p):
    nc = d.nc

    xpt = p.cpool.tile([48, M], BF16, name="xpt_sb", tag="xpt")
    nc.sync.dma_start(out=xpt, in_=d.xpt.ap())
    wp = p.cpool.tile([48, C], BF16, name="wp_sb", tag="wp")
    nc.sync.dma_start(out=wp, in_=d.wp.ap())
    pe = p.cpool.tile([128, CT, M], BF16, name="pe_sb", tag="pe")
    nc.sync.dma_start(out=pe, in_=d.pe.ap().transpose([1, 0, 2]))
    d.ctxt_sb = p.cpool.tile([128, CT, S], BF16, name="ctxt_sb", tag="ctxt")
    nc.sync.dma_start(out=d.ctxt_sb, in_=d.ctxt.ap().transpose([1, 0, 2]))
    d.sel_sb = p.cpool.tile([128, CT, 128], BF16, name="sel_sb", tag="sel")
    nc.sync.dma_start(out=d.sel_sb, in_=d.sel.ap().transpose([1, 0, 2]))
    d.wmask_sb = p.cpool.tile([128, 1024], BF16, name="wmask_sb",
                              tag="wmask")
    nc.sync.dma_start(out=d.wmask_sb, in_=d.wmask.ap())
    d.bh_sb = p.cpool.tile([48, 1], F32, name="bh_sb", tag="bh")
    nc.sync.dma_start(out=d.bh_sb, in_=d.bh.ap())
    for pref in ("s", "g"):
        t = p.cpool.tile([128, 24], F32, name=pref + "bias_sb",
                         tag=pref + "bias")
        nc.sync.dma_start(out=t, in_=getattr(d, pref + "bias").ap())
        setattr(d, pref + "bias_sb", t)

    ones_col = p.cpool.tile([128, 1], BF16, name="ones_col", tag="ones")
    nc.vector.memset(ones_col, 1.0)
    d.ones_col = ones_col
    epscol = p.cpool.tile([128, 1], F32, name="epscol", tag="eps")
    nc.vector.memset(epscol, EPS)
    d.epscol = epscol

    h = p.state.tile([128, CT, M], BF16, name="h", tag="hbuf")
    # patch embedding + (conv bias + positional) add
    for ct in range(CT):
        for ms in range(2):
            msl = slice(ms * 512, (ms + 1) * 512)
            ps = p.ps.tile([128, 512], F32, name="pe_ps", tag="ps")
            nc.tensor.matmul(ps, wp[:, ct * 128:(ct + 1) * 128],
                             xpt[:, msl], start=True, stop=True)
            nc.vector.tensor_add(h[:, ct, msl], ps, pe[:, ct, msl])
    if _STAGE <= 1:
        return _dbg_out(d, p, h[0:48, 0, :])

    _block(d, p, "s", h, 20)
    if _STAGE <= 2 or (20 <= _STAGE <= 25):
        if _STAGE <= 2:
            return _dbg_out(d, p, h[0:48, 0, :])
        return

    # permute tokens to m' (window-major for the 4x4x4 windows)
    hg = p.state.tile([128, CT, M], BF16, name="hg", tag="zbuf2")
    for ct in range(CT):
        for a in range(4):
            src = h[:, ct, :].rearrange("p (f ph pw) -> p f ph pw",
                                        f=4, ph=16)[:, :, 4 * a:4 * a + 4, :]
            src = src.rearrange("p f i (b j) -> p b f i j", b=4)
            nc.vector.tensor_copy(
                out=hg[:, ct, a * 256:(a + 1) * 256].rearrange(
                    "p (b f i j) -> p b f i j", b=4, f=4, i=4),
                in_=src)
    if _STAGE <= 3:
        return _dbg_out(d, p, hg[0:48, 0, :])

    gwb = _block(d, p, "g", hg, 30)
    if _STAGE <= 4 or (30 <= _STAGE <= 35):
        if _STAGE <= 4:
            return _dbg_out(d, p, hg[0:48, 0, :])
        return

    # output head (m' order; host undoes the permutation)
    outT = p.cpool.tile([48, M], F32, name="outT", tag="outT")
    for ms in range(2):
        msl = slice(ms * 512, (ms + 1) * 512)
        ps = p.ps.tile([128, 512], F32, name="hd_ps", tag="ps")
        for kt in range(CT):
            nc.tensor.matmul(ps[0:48, :],
                             gwb[:, kt, OB["wh"]:OB["wh"] + 48],
                             hg[:, kt, msl], start=(kt == 0), stop=(kt == 3))
        nc.scalar.activation(out=outT[:, msl], in_=ps[0:48, :],
                             func=AF.Identity, bias=d.bh_sb[:, 0:1],
                             scale=1.0)
    nc.sync.dma_start(out=d.out.ap(), in_=outT)


def _build_nc(loop_n=1):
    key = loop_n
    if key in _BUILD_CACHE:
        return _BUILD_CACHE[key]
    nc = bacc.Bacc("TRN2", target_bir_lowering=False, debug=False,
                   num_devices=8)
    d = _declare_inputs(nc)
    with tile.TileContext(nc) as tc:
        p = Pools()
        import contextlib
        stack = contextlib.ExitStack()
        with stack:
            p.ps = stack.enter_context(
                tc.tile_pool(name="ps", bufs=2, space="PSUM"))
            p.psA = stack.enter_context(
                tc.tile_pool(name="psA", bufs=1, space="PSUM"))
            p.psB = stack.enter_context(
                tc.tile_pool(name="psB", bufs=1, space="PSUM"))
            p.wpool = stack.enter_context(tc.tile_pool(name="wpool", bufs=2))
            p.cpool = stack.enter_context(tc.tile_pool(name="cpool", bufs=1))
            p.state = stack.enter_context(tc.tile_pool(name="state", bufs=1))
            p.lncomb = stack.enter_context(
                tc.tile_pool(name="lncomb", bufs=2))
            p.lncs = stack.enter_context(tc.tile_pool(name="lncs", bufs=2))
            p.lnar = stack.enter_context(tc.tile_pool(name="lnar", bufs=2))
            p.lnt = stack.enter_context(tc.tile_pool(name="lnt", bufs=3))
            p.ex = stack.enter_context(tc.tile_pool(name="ex", bufs=2))
            p.rb = stack.enter_context(tc.tile_pool(name="rb", bufs=2))
            p.rrow = stack.enter_context(tc.tile_pool(name="rrow", bufs=2))
            p.rec = stack.enter_context(tc.tile_pool(name="rec", bufs=2))
            p.rsb = stack.enter_context(tc.tile_pool(name="rsb", bufs=2))
            p.ktc = stack.enter_context(tc.tile_pool(name="ktc", bufs=1))
            p.vc = stack.enter_context(tc.tile_pool(name="vc", bufs=1))
            for _ in range(loop_n):
                _body(d, p)
    nc.compile()
    _BUILD_CACHE[key] = nc
    return nc


# ---------------------------------------------------------------- runner

_MPRIME = None
_RUNNERS = {}


class _Runner:
    """AOT-compiled executable + device-resident inputs for one nc."""

    def __init__(self, nc, n_cores=8):
        import jax
        from jax.sharding import Mesh, PartitionSpec
        try:
            from jax.experimental.shard_map import shard_map
        except ImportError:
            from jax import shard_map
        import concourse.bass2jax as b2j
        self.jax = jax
        b2j.install_neuronx_cc_hook()
        self.nc = nc
        self.n_cores = n_cores
        partition_name = (nc.partition_id_tensor.name
                          if nc.partition_id_tensor else None)
        in_names, out_names, out_avals, zero_outs = [], [], [], []
        for alloc in nc.m.functions[0].allocations:
            if not isinstance(alloc, mybir.MemoryLocationSet):
                continue
            name = alloc.memorylocations[0].name
            if alloc.kind == "ExternalInput":
                if name != partition_name:
                    in_names.append(name)
            elif alloc.kind == "ExternalOutput":
                out_names.append(name)
                shape = tuple(alloc.tensor_shape)
                dtype = mybir.dt.np(alloc.dtype)
                out_avals.append(jax.core.ShapedArray(shape, dtype))
                zero_outs.append(np.zeros(shape, dtype))
        self.in_names, self.out_names = in_names, out_names
        self.out_avals, self.zero_outs = out_avals, zero_outs
        n_params, n_outs = len(in_names), len(out_avals)
        all_in = list(in_names) + list(out_names)
        if partition_name is not None:
            all_in.append(partition_name)
        donate = tuple(range(n_params, n_params + n_outs))

        def _fn(*args):
            operands = list(args)
            if partition_name is not None:
                operands.append(b2j.partition_id_tensor())
            return tuple(b2j._bass_exec_p.bind(
                *operands, out_avals=tuple(out_avals),
                in_names=tuple(all_in), out_names=tuple(out_names),
                lowering_input_output_aliases=(),
                sim_require_finite=False, sim_require_nnan=False, nc=nc))

        devices = jax.devices()[:n_cores]
        self.mesh = Mesh(np.asarray(devices), ("core",))
        in_specs = (PartitionSpec("core"),) * (n_params + n_outs)
        out_specs = (PartitionSpec("core"),) * len(out_names)
        jitted = jax.jit(
            shard_map(_fn, mesh=self.mesh, in_specs=in_specs,
                      out_specs=out_specs, check_rep=False),
            donate_argnums=donate, keep_unused=True)
        arg_structs = []
        self._in_shapes = {}
        dummy = None
        self.compiled = None
        self._jitted = jitted
        self.sharding = jax.sharding.NamedSharding(
            self.mesh, PartitionSpec("core"))
        self.dev_inputs = None

    def _concat(self, in_maps):
        return [np.concatenate([np.asarray(m[n]) for m in in_maps], axis=0)
                for n in self.in_names]

    def _compile(self, concat_in):
        jax = self.jax
        zeros = [np.zeros((self.n_cores * z.shape[0], *z.shape[1:]), z.dtype)
                 for z in self.zero_outs]
        structs = [jax.ShapeDtypeStruct(a.shape, a.dtype)
                   for a in concat_in + zeros]
        self.compiled = self._jitted.lower(*structs).compile()
        self._zero_shapes = [(z.shape, z.dtype) for z in zeros]

    def put_inputs(self, in_maps):
        jax = self.jax
        concat_in = self._concat(in_maps)
        if self.compiled is None:
            self._compile(concat_in)
        self.dev_inputs = [jax.device_put(a, self.sharding)
                           for a in concat_in]
        jax.block_until_ready(self.dev_inputs)

    def _fresh_zeros(self):
        jax = self.jax
        zs = [jax.device_put(np.zeros(s, d), self.sharding)
              for s, d in self._zero_shapes]
        jax.block_until_ready(zs)
        return zs

    def execute(self, fetch=True):
        jax = self.jax
        outs = self.compiled(*self.dev_inputs, *self._fresh_zeros())
        jax.block_until_ready(outs)
        if not fetch:
            return None
        res = []
        for c in range(self.n_cores):
            m = {}
            for i, name in enumerate(self.out_names):
                full = np.asarray(outs[i])
                m[name] = full.reshape(self.n_cores,
                                       *self.out_avals[i].shape)[c]
            res.append(m)
        return res

    def execute_timed(self):
        """One timed execution: zeros staged outside, only dispatch+exec."""
        import time
        jax = self.jax
        zs = self._fresh_zeros()
        t0 = time.perf_counter()
        outs = self.compiled(*self.dev_inputs, *zs)
        jax.block_until_ready(outs)
        t1 = time.perf_counter()
        for o in outs:
            o.delete()
        return t1 - t0


def get_runner(loop_n=1):
    if loop_n not in _RUNNERS:
        _RUNNERS[loop_n] = _Runner(_build_nc(loop_n))
    return _RUNNERS[loop_n]


def kernel(**inputs) -> np.ndarray:
    global _MPRIME
    if _MPRIME is None:
        _MPRIME = _mprime_index()
    in_maps = host_prep(inputs)
    try:
        r = get_runner(1)
        r.put_inputs(in_maps)
        results = r.execute(fetch=True)
    except Exception:
        # fall back to the stock SPMD path if the AOT runner misbehaves
        res = run_bass_kernel_spmd(_build_nc(1), in_maps,
                                   core_ids=list(range(8)))
        results = res.results
    out = np.zeros((B, CIN, F, IMG, IMG), np.float32)
    for c in range(8):
        b, t = c // 4, c % 4
        ot = results[c]["out"]                     # [48, M] m'-order
        om = np.empty_like(ot)
        om[:, _MPRIME] = ot
        om = om.reshape(P, P, CIN, 4, PD, PD)
        om = om.transpose(2, 3, 4, 0, 5, 1)
        out[b, :, 4 * t:4 * t + 4] = om.reshape(CIN, 4, IMG, IMG)
    return out


if __name__ == "__main__":
    import reference
    import jax
    cpu = jax.devices("cpu")[0]
    with jax.default_device(cpu):
        ins = reference.setup_inputs()
        ins = {k: np.asarray(v) for k, v in ins.items()}
        exp = np.asarray(reference.reference(**ins))
    act = kernel(**ins)
    err = np.abs(act - exp).max() / (np.abs(exp).max() + 1e-12)
    print("Relative error:", err)


# revision 5
# speedup vs baseline: 1.0559x; 1.0559x over previous
# Trainium2 Bass kernel v2 for nn_DiT_89086211653924 (windowed-attention video
# DiT).  Same sharding as v1: core c -> (batch c//4, temporal group c%4), 1024
# tokens per core, zero collectives.
#
# v2 changes vs v1:
#  - all matmuls in bf16 (4x PE rate), activations stored bf16, psum fp32
#  - LayerNorm stats via gpsimd partition_all_reduce (13 ops vs ~30)
#  - attention softmax batched per window group: one exp per 4 heads (multi-
#    bank PSUM AP), denominators via ones-matmuls, reciprocal rows broadcast
#    via gpsimd partition_broadcast
#  - block-g (4x4x4 window) scores computed per window-PAIR as one
#    [128k, 128q] matmul per head; cross-window quadrants of exp(scores)
#    are zeroed by a 0/1 mask (exact block-diagonal attention, half the
#    score/ob matmuls)
#  - K-projection bias dropped (softmax-invariant), V-projection bias folded
#    into the out-projection bias on the host
#  - weights shipped as 4 packed tensors (2 per block), few big DMAs
#  - AOT-cached jit executable + on-device input caching so repeated calls
#    do not pay client-side recompile/transfer
import sys

sys.path.insert(0, "/opt/trn_rl_repo")

import numpy as np
import ml_dtypes

import concourse.bacc as bacc
import concourse.mybir as mybir
import concourse.tile as tile
import concourse.bass as bass
from concourse.bass_utils import run_bass_kernel_spmd

F32 = mybir.dt.float32
BF16 = mybir.dt.bfloat16
AF = mybir.ActivationFunctionType
ALU = mybir.AluOpType
ReduceOp = bass.bass_isa.ReduceOp
BFNP = ml_dtypes.bfloat16

B, CIN, F, IMG, P = 2, 3, 16, 64, 4
PD = IMG // P          # 16 patches per side
NP = PD * PD           # 256 patches per frame
C, NH = 512, 8
HD = C // NH           # 64
S = 16                 # action context length
SCALE = float(1.0 / np.sqrt(HD))
M = 1024               # tokens per core (4 frames)
CT = 4                 # c tiles of 128
EPS = 1e-5
PEN = 5.0              # sqrt of window penalty (exactly representable in bf16)

# weight column offsets inside each half-pack (units of C=512 columns)
# half A: wq1 wk1 wv1 wo1 wq2   half B: wk2 wv2 wo2 wf1 wf2 [wh]
OA = {"wq1": 0, "wk1": 512, "wv1": 1024, "wo1": 1536, "wq2": 2048}
OB = {"wk2": 0, "wv2": 512, "wo2": 1024, "wf1": 1536, "wf2": 2048, "wh": 2560}
NWA, NWB = 2560, 2608

_BUILD_CACHE = {}

import os as _os
_STAGE = int(_os.environ.get("DIT_STAGE", "99"))


# ---------------------------------------------------------------- host prep

def _w4(a):
    """[512, N] k-major weight -> [4, 128, N] bf16."""
    return np.ascontiguousarray(a.reshape(4, 128, -1)).astype(BFNP)


def _colscal(a):
    """[512] bias -> [128, 4] per-partition fp32 column layout."""
    return np.ascontiguousarray(a.reshape(4, 128).T.astype(np.float32))


def _mprime_index():
    a, bb, f, i, j = np.meshgrid(
        np.arange(4), np.arange(4), np.arange(4), np.arange(4), np.arange(4),
        indexing="ij")
    return (f * 256 + (4 * a + i) * 16 + 4 * bb + j).reshape(-1)


def _fold_block(p, w):
    g, b = w[p + "_ln_g"], w[p + "_ln_b"]
    qkv1, wo1, bo1 = w[p + "_qkv1"], w[p + "_wo1"], w[p + "_bo1"]
    qkv2, wo2, bo2 = w[p + "_qkv2"], w[p + "_wo2"], w[p + "_bo2"]
    f1w, f1b = w[p + "_fc1w"], w[p + "_fc1b"]
    f2w, f2b = w[p + "_fc2w"], w[p + "_fc2b"]
    d = {}
    d["wq1"] = (g[0][:, None] * qkv1[0]) * SCALE
    d["bq1"] = (b[0] @ qkv1[0]) * SCALE
    d["wk1"] = g[0][:, None] * qkv1[1]          # k bias dropped (softmax-inv)
    d["wv1"] = g[0][:, None] * qkv1[2]
    bv1 = b[0] @ qkv1[2]
    d["wo1"] = wo1
    d["bo1"] = bo1 + bv1 @ wo1                  # v bias folded through softmax
    d["wq2"] = (g[1][:, None] * qkv2[0]) * SCALE
    d["bq2"] = (b[1] @ qkv2[0]) * SCALE
    d["wk2"] = qkv2[1]
    d["wv2"] = qkv2[2]
    d["wo2"], d["bo2"] = wo2, bo2
    d["wf1"] = g[2][:, None] * f1w
    d["bf1"] = b[2] @ f1w + f1b
    d["wf2"], d["bf2"] = f2w, f2b
    return d


def host_prep(inputs):
    """Full inputs -> list of 8 per-core input maps (shared weights aliased)."""
    w = {k: np.asarray(v) for k, v in inputs.items()}
    x = w["x"].astype(np.float32)
    actions = np.asarray(w["actions"])

    shared = {}
    shared["wp"] = np.ascontiguousarray(
        w["conv_w"].reshape(C, CIN * P * P).T).astype(BFNP)       # [48, 512]
    shared["bh"] = np.ascontiguousarray(
        w["head_b"].reshape(48, 1).astype(np.float32))

    for p in ("s", "g"):
        fb = _fold_block(p, w)
        wa = np.concatenate(
            [_w4(fb[n]) for n in ("wq1", "wk1", "wv1", "wo1", "wq2")], axis=2)
        wbl = [_w4(fb[n]) for n in ("wk2", "wv2", "wo2", "wf1", "wf2")]
        if p == "g":
            wbl.append(_w4(w["head_w"]))                           # [4,128,48]
        else:
            wbl.append(np.zeros((4, 128, 48), BFNP))
        wb = np.concatenate(wbl, axis=2)
        shared[p + "wa"] = np.ascontiguousarray(wa)
        shared[p + "wb"] = np.ascontiguousarray(wb)
        shared[p + "bias"] = np.ascontiguousarray(np.concatenate(
            [_colscal(fb[n]) for n in ("bq1", "bo1", "bq2", "bo2",
                                       "bf1", "bf2")], axis=1))    # [128, 24]

    # cross-attn selector matrices: R[p, ct, q] = rec[32*((2ct+(p>=64))%4), q]
    sel = np.zeros((4, 128, 128), np.float32)
    for ct in range(4):
        for pp in range(128):
            h = 2 * ct + (1 if pp >= 64 else 0)
            sel[ct, 32 * (h % 4), pp] = 1.0
    shared["sel"] = sel.astype(BFNP)
    same = (np.arange(128)[:, None] < 64) == (np.arange(128)[None, :] < 64)
    shared["wmask"] = np.ascontiguousarray(
        np.tile(same.astype(np.float32), (1, 8))).astype(BFNP)

    # positional embedding folded with conv bias: pe[c, (f, ph, pw)]
    pe_all = (w["conv_b"][None, None, :] + w["t_pos"][:, None, :]
              + w["sp_pos"][None, :, :])                           # [F, NP, C]

    per_core = []
    for c in range(8):
        b, t = c // 4, c % 4
        m = dict(shared)
        xs = x[b, :, 4 * t:4 * t + 4].reshape(CIN, 4, PD, P, PD, P)
        xs = xs.transpose(0, 3, 5, 1, 2, 4)        # (c, p, q, f, ph, pw)
        m["xpt"] = np.ascontiguousarray(
            xs.reshape(CIN * P * P, M)).astype(BFNP)
        pe = pe_all[4 * t:4 * t + 4].reshape(M, C).T               # [C, M]
        m["pe"] = np.ascontiguousarray(pe.reshape(4, 128, M)).astype(BFNP)
        ctx = (w["act_table"][actions[b]] + w["act_pos"][0])
        m["ctxt"] = np.ascontiguousarray(
            ctx.T.reshape(4, 128, S)).astype(BFNP)
        per_core.append(m)
    return per_core


# ---------------------------------------------------------------- device

class Dev:
    pass


class Pools:
    pass


def _declare_inputs(nc):
    d = Dev()
    d.nc = nc
    mkb = lambda name, shape: nc.dram_tensor(name, shape, BF16,
                                             kind="ExternalInput")
    mkf = lambda name, shape: nc.dram_tensor(name, shape, F32,
                                             kind="ExternalInput")
    d.xpt = mkb("xpt", [48, M])
    d.pe = mkb("pe", [4, 128, M])
    d.ctxt = mkb("ctxt", [4, 128, S])
    d.wp = mkb("wp", [48, C])
    d.bh = mkf("bh", [48, 1])
    d.sel = mkb("sel", [4, 128, 128])
    d.wmask = mkb("wmask", [128, 1024])
    for p in ("s", "g"):
        setattr(d, p + "wa", mkb(p + "wa", [4, 128, NWA]))
        setattr(d, p + "wb", mkb(p + "wb", [4, 128, NWB]))
        setattr(d, p + "bias", mkf(p + "bias", [128, 24]))
    d.out = nc.dram_tensor("out", [48, M], F32, kind="ExternalOutput")
    return d


def _dbg_out(d, p, t_sb):
    nc = d.nc
    outT = p.cpool.tile([48, M], F32, name="outT", tag="outT")
    nc.vector.tensor_copy(out=outT, in_=t_sb)
    nc.sync.dma_start(out=d.out.ap(), in_=outT)


def _load_whalf(d, p, dram, name):
    nc = d.nc
    n = dram.shape[2]
    t = p.wpool.tile([128, 4, n], BF16, name=name, tag="whalf")
    nc.sync.dma_start(out=t, in_=dram.ap().transpose([1, 0, 2]))
    return t


def _linear_fm(d, p, w_sb, col0, x_sb, out_sb, mode, bias_col=None):
    """Feature-major linear over tokens.  mode: 'act' (ACT identity+bias),
    'copy' (DVE copy, no bias), 'res' (h += ps + bias), 'gelu'."""
    nc = d.nc
    for ct in range(CT):
        for ms in range(2):
            ps = p.ps.tile([128, 512], F32, name="lin_ps", tag="ps")
            for kt in range(CT):
                nc.tensor.matmul(
                    ps, w_sb[:, kt, col0 + ct * 128:col0 + (ct + 1) * 128],
                    x_sb[:, kt, ms * 512:(ms + 1) * 512],
                    start=(kt == 0), stop=(kt == 3))
            sl = out_sb[:, ct, ms * 512:(ms + 1) * 512]
            if mode == "res":
                nc.vector.scalar_tensor_tensor(
                    out=sl, in0=ps, scalar=bias_col[:, ct:ct + 1], in1=sl,
                    op0=ALU.add, op1=ALU.add)
            elif mode == "copy":
                nc.vector.tensor_copy(out=sl, in_=ps)
            elif mode == "act":
                nc.scalar.activation(out=sl, in_=ps, func=AF.Identity,
                                     bias=bias_col[:, ct:ct + 1], scale=1.0)
            elif mode == "gelu":
                nc.scalar.activation(out=sl, in_=ps, func=AF.Gelu,
                                     bias=bias_col[:, ct:ct + 1], scale=1.0)


def _v_proj(d, p, w_sb, col0, x_sb, v_sb):
    """Token-major value projection v_sb[128 tok, mt, C] (no bias)."""
    nc = d.nc
    for mt in range(8):
        ps = p.ps.tile([128, 512], F32, name="v_ps", tag="ps")
        for kt in range(CT):
            nc.tensor.matmul(
                ps, x_sb[:, kt, mt * 128:(mt + 1) * 128],
                w_sb[:, kt, col0:col0 + 512],
                start=(kt == 0), stop=(kt == 3))
        nc.scalar.copy(out=v_sb[:, mt, :], in_=ps)


def _ln(d, p, h_sb, z_sb):
    """z = (h - mean_c) * rsqrt(var_c + eps) via gpsimd partition allreduce."""
    nc = d.nc
    sq = p.state.tile([128, CT, M], BF16, name="ln_sq", tag="scratch")
    nc.scalar.activation(out=sq, in_=h_sb, func=AF.Square)
    c2 = p.lncomb.tile([128, 2, M], BF16, name="ln_c2", tag="lncomb")
    nc.vector.tensor_add(c2, h_sb[:, 0:2, :], h_sb[:, 2:4, :])
    cs = p.lncs.tile([128, M], BF16, name="ln_cs", tag="lncs")
    nc.vector.tensor_add(cs, c2[:, 0, :], c2[:, 1, :])
    d2 = p.lncomb.tile([128, 2, M], BF16, name="ln_d2", tag="lncomb")
    nc.vector.tensor_add(d2, sq[:, 0:2, :], sq[:, 2:4, :])
    ds_ = p.lncs.tile([128, M], BF16, name="ln_ds", tag="lncs")
    nc.vector.tensor_add(ds_, d2[:, 0, :], d2[:, 1, :])
    sh = p.lnar.tile([128, M], BF16, name="ln_sh", tag="lnar")
    nc.gpsimd.partition_all_reduce(sh, cs, 128, ReduceOp.add)
    sqs = p.lnar.tile([128, M], BF16, name="ln_sqs", tag="lnar")
    nc.gpsimd.partition_all_reduce(sqs, ds_, 128, ReduceOp.add)
    mu2 = p.lnt.tile([128, M], BF16, name="ln_mu2", tag="lnt")
    nc.vector.tensor_mul(mu2, sh, sh)
    varu = p.lnt.tile([128, M], BF16, name="ln_varu", tag="lnt")
    # varu = mu2/512 - sumsq  (= -512*var)
    nc.vector.scalar_tensor_tensor(out=varu, in0=mu2, scalar=1.0 / C,
                                   in1=sqs, op0=ALU.mult, op1=ALU.subtract)
    sd = p.lnt.tile([128, M], BF16, name="ln_sd", tag="lnt")
    nc.scalar.activation(out=sd, in_=varu, func=AF.Sqrt,
                         bias=d.epscol, scale=-1.0 / C)
    r = p.lnt.tile([128, M], BF16, name="ln_r", tag="lnt")
    with nc.allow_low_precision(reason="rel tolerance 2e-2"):
        nc.vector.reciprocal(r, sd)
    psi = p.lnt.tile([128, M], BF16, name="ln_psi", tag="lnt")
    nc.vector.scalar_tensor_tensor(out=psi, in0=sh, scalar=-1.0 / C,
                                   in1=r, op0=ALU.mult, op1=ALU.mult)
    rb = r.unsqueeze(1).broadcast_to([128, CT, M])
    pb = psi.unsqueeze(1).broadcast_to([128, CT, M])
    nc.vector.tensor_mul(z_sb, h_sb, rb)
    nc.vector.tensor_add(z_sb, z_sb, pb)


def _attn_s(d, p, qt, kt, v_sb, attn):
    """Block-s self attention: 4 windows (frames) x 256 tokens, 8 heads."""
    nc = d.nc
    for w in range(4):
        exs = []
        for g in range(2):
            scg = p.psA.tile([128, 4, 512], F32, name="scg_s", tag="psA")
            for hh in range(4):
                h_ = 4 * g + hh
                hp, ct = 64 * (h_ % 2), h_ // 2
                for st in range(2):
                    nc.tensor.matmul(
                        scg[:, hh, st * 256:(st + 1) * 256],
                        kt[hp:hp + 64, ct,
                           w * 256 + st * 128:w * 256 + st * 128 + 128],
                        qt[hp:hp + 64, ct, w * 256:(w + 1) * 256],
                        start=True, stop=True, tile_position=(hp, 0))
            ex = p.ex.tile([128, 4, 2, 256], BF16, name="ex_s", tag="ex")
            nc.scalar.activation(out=ex, in_=scg, func=AF.Exp)
            exs.append(ex)
        dn = p.psA.tile([1, 2048], F32, name="dn_s", tag="psA")
        for h_ in range(8):
            g, hh = h_ // 4, h_ % 4
            for st in range(2):
                nc.tensor.matmul(
                    dn[0:1, h_ * 256:(h_ + 1) * 256],
                    d.ones_col, exs[g][:, hh, st, :],
                    start=(st == 0), stop=(st == 1))
        rrow = p.rrow.tile([1, 2048], BF16, name="rr_s", tag="rrow")
        with nc.allow_low_precision(reason="rel tolerance 2e-2"):
            nc.vector.reciprocal(rrow, dn)
        rb = p.rb.tile([128, 2048], BF16, name="rb_s", tag="rb")
        nc.gpsimd.partition_broadcast(rb, rrow, 128)
        rbv = rb.rearrange("p (h q) -> p h q", h=8)
        for g in range(2):
            in1 = rbv[:, 4 * g:4 * g + 4, :].unsqueeze(2).broadcast_to(
                [128, 4, 2, 256])
            nc.vector.tensor_mul(exs[g], exs[g], in1)
        ob = p.psB.tile([128, 4, 256], F32, name="ob_s", tag="psB")
        for h_ in range(8):
            g, hh = h_ // 4, h_ % 4
            hp, ct = 64 * (h_ % 2), h_ // 2
            for st in range(2):
                nc.tensor.matmul(
                    ob[hp:hp + 64, ct, :],
                    v_sb[:, 2 * w + st, h_ * 64:h_ * 64 + 64],
                    exs[g][:, hh, st, :],
                    start=(st == 0), stop=(st == 1), tile_position=(0, hp))
        nc.scalar.copy(out=attn[:, :, w * 256:(w + 1) * 256], in_=ob)


def _attn_g(d, p, qt, kt, v_sb, attn):
    """Block-g self attention: 16 windows x 64 tokens processed as 8 window
    pairs; cross-window terms masked to exactly 0 after exp.

    Scores are banked by head parity (slice idx = 4*(h%2) + h//2) so all
    concurrent matmuls in a PSUM bank share one contraction row group."""
    nc = d.nc
    for wp in range(8):
        scg = p.psB.tile([128, 8, 128], F32, name="scg_g", tag="psB")
        for h_ in range(8):
            hp, ct = 64 * (h_ % 2), h_ // 2
            idx = 4 * (h_ % 2) + ct
            nc.tensor.matmul(
                scg[:, idx, :],
                kt[hp:hp + 64, ct, wp * 128:(wp + 1) * 128],
                qt[hp:hp + 64, ct, wp * 128:(wp + 1) * 128],
                start=True, stop=True, tile_position=(hp, 0))
        ex = p.ex.tile([128, 8, 128], BF16, name="ex_g", tag="ex")
        nc.scalar.activation(out=ex, in_=scg, func=AF.Exp)
        # zero the cross-window quadrants (exact block-diagonal attention)
        nc.vector.tensor_mul(
            ex, ex, d.wmask_sb.rearrange("p (h q) -> p h q", h=8))
        # psA is idle during block-g self-attention; using it for the
        # denominators lets the next window pair's scores start in psB
        # while this pair's normalization is still in flight.
        dn = p.psA.tile([1, 1024], F32, name="dn_g", tag="psA")
        for half in range(2):
            nc.tensor.matmul(
                dn[0:1, half * 512:(half + 1) * 512],
                d.ones_col, ex[:, 4 * half:4 * half + 4, :].rearrange(
                    "p a b -> p (a b)"),
                start=True, stop=True)
        rrow = p.rrow.tile([1, 1024], BF16, name="rr_g", tag="rrow")
        with nc.allow_low_precision(reason="rel tolerance 2e-2"):
            nc.vector.reciprocal(rrow, dn)
        rb = p.rb.tile([128, 1024], BF16, name="rb_g", tag="rb")
        nc.gpsimd.partition_broadcast(rb, rrow, 128)
        nc.vector.tensor_mul(ex, ex, rb.rearrange("p (h q) -> p h q", h=8))
        ob = p.ps.tile([128, 4, 128], F32, name="ob_g", tag="ps")
        for h_ in range(8):
            hp, ct = 64 * (h_ % 2), h_ // 2
            idx = 4 * (h_ % 2) + ct
            nc.tensor.matmul(
                ob[hp:hp + 64, ct, :],
                v_sb[:, wp, h_ * 64:h_ * 64 + 64],
                ex[:, idx, :],
                start=True, stop=True, tile_position=(0, hp))
        nc.scalar.copy(out=attn[:, :, wp * 128:(wp + 1) * 128], in_=ob)


def _cross_attn(d, p, prefix, wb_sb, qt2, attn):
    """Cross attention to the 16 action-context tokens."""
    nc = d.nc
    # ctx K feature-major [128, 4, 16]
    ktc = p.ktc.tile([128, CT, S], BF16, name=prefix + "ktc", tag="ktc")
    for ct in range(CT):
        ps = p.ps.tile([128, 512], F32, name="ktc_ps", tag="ps")
        for kt in range(CT):
            nc.tensor.matmul(
                ps[:, 0:S],
                wb_sb[:, kt, OB["wk2"] + ct * 128:OB["wk2"] + (ct + 1) * 128],
                d.ctxt_sb[:, kt, :], start=(kt == 0), stop=(kt == 3))
        nc.scalar.copy(out=ktc[:, ct, :], in_=ps[:, 0:S])
    # ctx V token-major [16, 512], replicated to the four 32-strips
    vc = p.vc.tile([128, C], BF16, name=prefix + "vc", tag="vc")
    ps = p.ps.tile([128, 512], F32, name="vc_ps", tag="ps")
    for kt in range(CT):
        nc.tensor.matmul(ps[0:S, :], d.ctxt_sb[:, kt, :],
                         wb_sb[:, kt, OB["wv2"]:OB["wv2"] + 512],
                         start=(kt == 0), stop=(kt == 3))
    nc.scalar.copy(out=vc[0:S, :], in_=ps[0:S, :])
    for q in range(1, 4):
        nc.sync.dma_start(out=vc[32 * q:32 * q + S, :], in_=vc[0:S, :])

    for ms in range(2):
        msl = slice(ms * 512, (ms + 1) * 512)
        sc = p.psB.tile([128, 2, 512], F32, name="sc_x", tag="psB")
        for h_ in range(8):
            g, hh = h_ // 4, h_ % 4
            hp, ct = 64 * (h_ % 2), h_ // 2
            nc.tensor.matmul(
                sc[32 * hh:32 * hh + S, g, :],
                ktc[hp:hp + 64, ct, :],
                qt2[hp:hp + 64, ct, msl],
                start=True, stop=True, tile_position=(hp, 32 * hh))
        exc = p.ex.tile([128, 2, 512], BF16, name="ex_x", tag="ex")
        nc.scalar.activation(out=exc, in_=sc, func=AF.Exp)
        dn = p.psB.tile([128, 2, 512], F32, name="dn_x", tag="psB")
        nc.vector.memset(dn, 1.0)
        for h_ in range(8):
            g, hh = h_ // 4, h_ % 4
            nc.tensor.matmul(
                dn[32 * hh:32 * hh + 1, g, :],
                d.ones_col[32 * hh:32 * hh + S, 0:1],
                exc[32 * hh:32 * hh + S, g, :],
                start=True, stop=True, skip_group_check=True,
                tile_position=(32 * hh, 32 * hh))
        rec = p.rec.tile([128, 2, 512], BF16, name="rec_x", tag="rec")
        with nc.allow_low_precision(reason="rel tolerance 2e-2"):
            nc.vector.reciprocal(rec, dn)
        R = p.psA.tile([128, 4, 512], F32, name="R_x", tag="psA")
        for ct in range(CT):
            nc.tensor.matmul(R[:, ct, :], d.sel_sb[:, ct, :],
                             rec[:, ct // 2, :], start=True, stop=True)
        rsb = p.rsb.tile([128, 4, 512], BF16, name="rsb_x", tag="rsb")
        nc.vector.tensor_copy(out=rsb, in_=R)
        obp = p.psA.tile([128, 4, 512], F32, name="ob_x", tag="psA")
        for h_ in range(8):
            g, hh = h_ // 4, h_ % 4
            hp, ct = 64 * (h_ % 2), h_ // 2
            nc.tensor.matmul(
                obp[hp:hp + 64, ct, :],
                vc[32 * hh:32 * hh + S, h_ * 64:h_ * 64 + 64],
                exc[32 * hh:32 * hh + S, g, :],
                start=True, stop=True, tile_position=(32 * hh, hp))
        nc.vector.tensor_mul(attn[:, :, msl], obp, rsb)


def _block(d, p, prefix, h_sb, dbg_base):
    nc = d.nc
    wa = _load_whalf(d, p, getattr(d, prefix + "wa"), prefix + "wa")
    z = p.state.tile([128, CT, M], BF16, name="z", tag="zbuf")
    attn = p.state.tile([128, CT, M], BF16, name="attn", tag="attnbuf")
    qt = p.state.tile([128, CT, M], BF16, name="qt", tag="qbuf")
    kt = p.state.tile([128, CT, M], BF16, name="kt", tag="kbuf")
    bias = getattr(d, prefix + "bias_sb")

    _ln(d, p, h_sb, z)
    if _STAGE == dbg_base + 0:
        return _dbg_out(d, p, z[0:48, 0, :])
    _linear_fm(d, p, wa, OA["wq1"], z, qt, "act", bias[:, 0:4])
    _linear_fm(d, p, wa, OA["wk1"], z, kt, "copy")
    v_sb = p.state.tile([128, 8, C], BF16, name="v", tag="vbuf")
    _v_proj(d, p, wa, OA["wv1"], z, v_sb)
    if _STAGE == dbg_base + 1:
        return _dbg_out(d, p, qt[0:48, 0, :])
    if prefix == "s":
        _attn_s(d, p, qt, kt, v_sb, attn)
    else:
        _attn_g(d, p, qt, kt, v_sb, attn)
    if _STAGE == dbg_base + 2:
        return _dbg_out(d, p, attn[0:48, 0, :])
    _linear_fm(d, p, wa, OA["wo1"], attn, h_sb, "res", bias[:, 4:8])

    if _STAGE == dbg_base + 3:
        return _dbg_out(d, p, h_sb[0:48, 0, :])
    _ln(d, p, h_sb, z)
    qt2 = p.state.tile([128, CT, M], BF16, name="qt2", tag="kbuf")
    _linear_fm(d, p, wa, OA["wq2"], z, qt2, "act", bias[:, 8:12])
    wb = _load_whalf(d, p, getattr(d, prefix + "wb"), prefix + "wb")
    _cross_attn(d, p, prefix, wb, qt2, attn)
    if _STAGE == dbg_base + 4:
        return _dbg_out(d, p, attn[0:48, 0, :])
    _linear_fm(d, p, wb, OB["wo2"], attn, h_sb, "res", bias[:, 12:16])
    if _STAGE == dbg_base + 5:
        return _dbg_out(d, p, h_sb[0:48, 0, :])

    _ln(d, p, h_sb, z)
    gbuf = p.state.tile([128, CT, M], BF16, name="gbuf", tag="scratch")
    _linear_fm(d, p, wb, OB["wf1"], z, gbuf, "gelu", bias[:, 16:20])
    _linear_fm(d, p, wb, OB["wf2"], gbuf, h_sb, "res", bias[:, 20:24])
    return wb


def _body(d, p):
    nc = d.nc

    xpt = p.cpool.tile([48, M], BF16, name="xpt_sb", tag="xpt")
    nc.sync.dma_start(out=xpt, in_=d.xpt.ap())
    wp = p.cpool.tile([48, C], BF16, name="wp_sb", tag="wp")
    nc.sync.dma_start(out=wp, in_=d.wp.ap())
    pe = p.cpool.tile([128, CT, M], BF16, name="pe_sb", tag="pe")
    nc.sync.dma_start(out=pe, in_=d.pe.ap().transpose([1, 0, 2]))
    d.ctxt_sb = p.cpool.tile([128, CT, S], BF16, name="ctxt_sb", tag="ctxt")
    nc.sync.dma_start(out=d.ctxt_sb, in_=d.ctxt.ap().transpose([1, 0, 2]))
    d.sel_sb = p.cpool.tile([128, CT, 128], BF16, name="sel_sb", tag="sel")
    nc.sync.dma_start(out=d.sel_sb, in_=d.sel.ap().transpose([1, 0, 2]))
    d.wmask_sb = p.cpool.tile([128, 1024], BF16, name="wmask_sb",
                              tag="wmask")
    nc.sync.dma_start(out=d.wmask_sb, in_=d.wmask.ap())
    d.bh_sb = p.cpool.tile([48, 1], F32, name="bh_sb", tag="bh")
    nc.sync.dma_start(out=d.bh_sb, in_=d.bh.ap())
    for pref in ("s", "g"):
        t = p.cpool.tile([128, 24], F32, name=pref + "bias_sb",
                         tag=pref + "bias")
        nc.sync.dma_start(out=t, in_=getattr(d, pref + "bias").ap())
        setattr(d, pref + "bias_sb", t)

    ones_col = p.cpool.tile([128, 1], BF16, name="ones_col", tag="ones")
    nc.vector.memset(ones_col, 1.0)
    d.ones_col = ones_col
    epscol = p.cpool.tile([128, 1], F32, name="epscol", tag="eps")
    nc.vector.memset(epscol, EPS)
    d.epscol = epscol

    h = p.state.tile([128, CT, M], BF16, name="h", tag="hbuf")
    # patch embedding + (conv bias + positional) add
    for ct in range(CT):
        for ms in range(2):
            msl = slice(ms * 512, (ms + 1) * 512)
            ps = p.ps.tile([128, 512], F32, name="pe_ps", tag="ps")
            nc.tensor.matmul(ps, wp[:, ct * 128:(ct + 1) * 128],
                             xpt[:, msl], start=True, stop=True)
            nc.vector.tensor_add(h[:, ct, msl], ps, pe[:, ct, msl])
    if _STAGE <= 1:
        return _dbg_out(d, p, h[0:48, 0, :])

    _block(d, p, "s", h, 20)
    if _STAGE <= 2 or (20 <= _STAGE <= 25):
        if _STAGE <= 2:
            return _dbg_out(d, p, h[0:48, 0, :])
        return

    # permute tokens to m' (window-major for the 4x4x4 windows)
    hg = p.state.tile([128, CT, M], BF16, name="hg", tag="zbuf2")
    for ct in range(CT):
        for a in range(4):
            src = h[:, ct, :].rearrange("p (f ph pw) -> p f ph pw",
                                        f=4, ph=16)[:, :, 4 * a:4 * a + 4, :]
            src = src.rearrange("p f i (b j) -> p b f i j", b=4)
            nc.vector.tensor_copy(
                out=hg[:, ct, a * 256:(a + 1) * 256].rearrange(
                    "p (b f i j) -> p b f i j", b=4, f=4, i=4),
                in_=src)
    if _STAGE <= 3:
        return _dbg_out(d, p, hg[0:48, 0, :])

    gwb = _block(d, p, "g", hg, 30)
    if _STAGE <= 4 or (30 <= _STAGE <= 35):
        if _STAGE <= 4:
            return _dbg_out(d, p, hg[0:48, 0, :])
        return

    # output head (m' order; host undoes the permutation)
    outT = p.cpool.tile([48, M], F32, name="outT", tag="outT")
    for ms in range(2):
        msl = slice(ms * 512, (ms + 1) * 512)
        ps = p.ps.tile([128, 512], F32, name="hd_ps", tag="ps")
        for kt in range(CT):
            nc.tensor.matmul(ps[0:48, :],
                             gwb[:, kt, OB["wh"]:OB["wh"] + 48],
                             hg[:, kt, msl], start=(kt == 0), stop=(kt == 3))
        nc.scalar.activation(out=outT[:, msl], in_=ps[0:48, :],
                             func=AF.Identity, bias=d.bh_sb[:, 0:1],
                             scale=1.0)
    nc.sync.dma_start(out=d.out.ap(), in_=outT)


def _build_nc(loop_n=1):
    key = loop_n
    if key in _BUILD_CACHE:
        return _BUILD_CACHE[key]
    nc = bacc.Bacc("TRN2", target_bir_lowering=False, debug=False,
                   num_devices=8)
    d = _declare_inputs(nc)
    with tile.TileContext(nc) as tc:
        p = Pools()
        import contextlib
        stack = contextlib.ExitStack()
        with stack:
            p.ps = stack.enter_context(
                tc.tile_pool(name="ps", bufs=2, space="PSUM"))
            p.psA = stack.enter_context(
                tc.tile_pool(name="psA", bufs=1, space="PSUM"))
            p.psB = stack.enter_context(
                tc.tile_pool(name="psB", bufs=1, space="PSUM"))
            p.wpool = stack.enter_context(tc.tile_pool(name="wpool", bufs=2))
            p.cpool = stack.enter_context(tc.tile_pool(name="cpool", bufs=1))
            p.state = stack.enter_context(tc.tile_pool(name="state", bufs=1))
            p.lncomb = stack.enter_context(
                tc.tile_pool(name="lncomb", bufs=2))
            p.lncs = stack.enter_context(tc.tile_pool(name="lncs", bufs=2))
            p.lnar = stack.enter_context(tc.tile_pool(name="lnar", bufs=2))
            p.lnt = stack.enter_context(tc.tile_pool(name="lnt", bufs=3))
            p.ex = stack.enter_context(tc.tile_pool(name="ex", bufs=2))
            p.rb = stack.enter_context(tc.tile_pool(name="rb", bufs=2))
            p.rrow = stack.enter_context(tc.tile_pool(name="rrow", bufs=2))
            p.rec = stack.enter_context(tc.tile_pool(name="rec", bufs=2))
            p.rsb = stack.enter_context(tc.tile_pool(name="rsb", bufs=2))
            p.ktc = stack.enter_context(tc.tile_pool(name="ktc", bufs=1))
            p.vc = stack.enter_context(tc.tile_pool(name="vc", bufs=1))
            for _ in range(loop_n):
                _body(d, p)
    nc.compile()
    _BUILD_CACHE[key] = nc
    return nc


# ---------------------------------------------------------------- runner

_MPRIME = None
_RUNNERS = {}


class _Runner:
    """AOT-compiled executable + device-resident inputs for one nc."""

    def __init__(self, nc, n_cores=8):
        import jax
        from jax.sharding import Mesh, PartitionSpec
        try:
            from jax.experimental.shard_map import shard_map
        except ImportError:
            from jax import shard_map
        import concourse.bass2jax as b2j
        self.jax = jax
        b2j.install_neuronx_cc_hook()
        self.nc = nc
        self.n_cores = n_cores
        partition_name = (nc.partition_id_tensor.name
                          if nc.partition_id_tensor else None)
        in_names, out_names, out_avals, zero_outs = [], [], [], []
        for alloc in nc.m.functions[0].allocations:
            if not isinstance(alloc, mybir.MemoryLocationSet):
                continue
            name = alloc.memorylocations[0].name
            if alloc.kind == "ExternalInput":
                if name != partition_name:
                    in_names.append(name)
            elif alloc.kind == "ExternalOutput":
                out_names.append(name)
                shape = tuple(alloc.tensor_shape)
                dtype = mybir.dt.np(alloc.dtype)
                out_avals.append(jax.core.ShapedArray(shape, dtype))
                zero_outs.append(np.zeros(shape, dtype))
        self.in_names, self.out_names = in_names, out_names
        self.out_avals, self.zero_outs = out_avals, zero_outs
        n_params, n_outs = len(in_names), len(out_avals)
        all_in = list(in_names) + list(out_names)
        if partition_name is not None:
            all_in.append(partition_name)
        donate = tuple(range(n_params, n_params + n_outs))

        def _fn(*args):
            operands = list(args)
            if partition_name is not None:
                operands.append(b2j.partition_id_tensor())
            return tuple(b2j._bass_exec_p.bind(
                *operands, out_avals=tuple(out_avals),
                in_names=tuple(all_in), out_names=tuple(out_names),
                lowering_input_output_aliases=(),
                sim_require_finite=False, sim_require_nnan=False, nc=nc))

        devices = jax.devices()[:n_cores]
        self.mesh = Mesh(np.asarray(devices), ("core",))
        in_specs = (PartitionSpec("core"),) * (n_params + n_outs)
        out_specs = (PartitionSpec("core"),) * len(out_names)
        jitted = jax.jit(
            shard_map(_fn, mesh=self.mesh, in_specs=in_specs,
                      out_specs=out_specs, check_rep=False),
            donate_argnums=donate, keep_unused=True)
        arg_structs = []
        self._in_shapes = {}
        dummy = None
        self.compiled = None
        self._jitted = jitted
        self.sharding = jax.sharding.NamedSharding(
            self.mesh, PartitionSpec("core"))
        self.dev_inputs = None

    def _concat(self, in_maps):
        return [np.concatenate([np.asarray(m[n]) for m in in_maps], axis=0)
                for n in self.in_names]

    def _compile(self, concat_in):
        jax = self.jax
        zeros = [np.zeros((self.n_cores * z.shape[0], *z.shape[1:]), z.dtype)
                 for z in self.zero_outs]
        structs = [jax.ShapeDtypeStruct(a.shape, a.dtype)
                   for a in concat_in + zeros]
        self.compiled = self._jitted.lower(*structs).compile()
        self._zero_shapes = [(z.shape, z.dtype) for z in zeros]

    def put_inputs(self, in_maps):
        jax = self.jax
        concat_in = self._concat(in_maps)
        if self.compiled is None:
            self._compile(concat_in)
        self.dev_inputs = [jax.device_put(a, self.sharding)
                           for a in concat_in]
        jax.block_until_ready(self.dev_inputs)

    def _fresh_zeros(self):
        jax = self.jax
        zs = [jax.device_put(np.zeros(s, d), self.sharding)
              for s, d in self._zero_shapes]
        jax.block_until_ready(zs)
        return zs

    def execute(self, fetch=True):
        jax = self.jax
        outs = self.compiled(*self.dev_inputs, *self._fresh_zeros())
        jax.block_until_ready(outs)
        if not fetch:
            return None
        res = []
        for c in range(self.n_cores):
            m = {}
            for i, name in enumerate(self.out_names):
                full = np.asarray(outs[i])
                m[name] = full.reshape(self.n_cores,
                                       *self.out_avals[i].shape)[c]
            res.append(m)
        return res

    def execute_timed(self):
        """One timed execution: zeros staged outside, only dispatch+exec."""
        import time
        jax = self.jax
        zs = self._fresh_zeros()
        t0 = time.perf_counter()
        outs = self.compiled(*self.dev_inputs, *zs)
        jax.block_until_ready(outs)
        t1 = time.perf_counter()
        for o in outs:
            o.delete()
        return t1 - t0


def get_runner(loop_n=1):
    if loop_n not in _RUNNERS:
        _RUNNERS[loop_n] = _Runner(_build_nc(loop_n))
    return _RUNNERS[loop_n]


def kernel(**inputs) -> np.ndarray:
    global _MPRIME
    if _MPRIME is None:
        _MPRIME = _mprime_index()
    in_maps = host_prep(inputs)
    try:
        r = get_runner(1)
        r.put_inputs(in_maps)
        results = r.execute(fetch=True)
    except Exception:
        # fall back to the stock SPMD path if the AOT runner misbehaves
        res = run_bass_kernel_spmd(_build_nc(1), in_maps,
                                   core_ids=list(range(8)))
        results = res.results
    out = np.zeros((B, CIN, F, IMG, IMG), np.float32)
    for c in range(8):
        b, t = c // 4, c % 4
        ot = results[c]["out"]                     # [48, M] m'-order
        om = np.empty_like(ot)
        om[:, _MPRIME] = ot
        om = om.reshape(P, P, CIN, 4, PD, PD)
        om = om.transpose(2, 3, 4, 0, 5, 1)
        out[b, :, 4 * t:4 * t + 4] = om.reshape(CIN, 4, IMG, IMG)
    return out


if __name__ == "__main__":
    import reference
    import jax
    cpu = jax.devices("cpu")[0]
    with jax.default_device(cpu):
        ins = reference.setup_inputs()
        ins = {k: np.asarray(v) for k, v in ins.items()}
        exp = np.asarray(reference.reference(**ins))
    act = kernel(**ins)
    err = np.abs(act - exp).max() / (np.abs(exp).max() + 1e-12)
    print("Relative error:", err)


# revision 9
# speedup vs baseline: 1.0815x; 1.0243x over previous
# Trainium2 Bass kernel v2 for nn_DiT_89086211653924 (windowed-attention video
# DiT).  Same sharding as v1: core c -> (batch c//4, temporal group c%4), 1024
# tokens per core, zero collectives.
#
# v2 changes vs v1:
#  - all matmuls in bf16 (4x PE rate), activations stored bf16, psum fp32
#  - LayerNorm stats via gpsimd partition_all_reduce (13 ops vs ~30)
#  - attention softmax batched per window group: one exp per 4 heads (multi-
#    bank PSUM AP), denominators via ones-matmuls, reciprocal rows broadcast
#    via gpsimd partition_broadcast
#  - block-g (4x4x4 window) scores computed per window-PAIR as one
#    [128k, 128q] matmul per head; cross-window quadrants of exp(scores)
#    are zeroed by a 0/1 mask (exact block-diagonal attention, half the
#    score/ob matmuls)
#  - K-projection bias dropped (softmax-invariant), V-projection bias folded
#    into the out-projection bias on the host
#  - weights shipped as 4 packed tensors (2 per block), few big DMAs
#  - AOT-cached jit executable + on-device input caching so repeated calls
#    do not pay client-side recompile/transfer
import sys

sys.path.insert(0, "/opt/trn_rl_repo")

import numpy as np
import ml_dtypes

import concourse.bacc as bacc
import concourse.mybir as mybir
import concourse.tile as tile
import concourse.bass as bass
from concourse.bass_utils import run_bass_kernel_spmd

F32 = mybir.dt.float32
BF16 = mybir.dt.bfloat16
AF = mybir.ActivationFunctionType
ALU = mybir.AluOpType
ReduceOp = bass.bass_isa.ReduceOp
BFNP = ml_dtypes.bfloat16

B, CIN, F, IMG, P = 2, 3, 16, 64, 4
PD = IMG // P          # 16 patches per side
NP = PD * PD           # 256 patches per frame
C, NH = 512, 8
HD = C // NH           # 64
S = 16                 # action context length
SCALE = float(1.0 / np.sqrt(HD))
M = 1024               # tokens per core (4 frames)
CT = 4                 # c tiles of 128
EPS = 1e-5
PEN = 5.0              # sqrt of window penalty (exactly representable in bf16)

# weight column offsets inside each half-pack (units of C=512 columns)
# half A: wq1 wk1 wv1 wo1 wq2   half B: wk2 wv2 wo2 wf1 wf2 [wh]
OA = {"wq1": 0, "wk1": 512, "wv1": 1024, "wo1": 1536, "wq2": 2048}
OB = {"wk2": 0, "wv2": 512, "wo2": 1024, "wf1": 1536, "wf2": 2048, "wh": 2560}
NWA, NWB = 2560, 2608

_BUILD_CACHE = {}

import os as _os
_STAGE = int(_os.environ.get("DIT_STAGE", "99"))


# ---------------------------------------------------------------- host prep

def _w4(a):
    """[512, N] k-major weight -> [4, 128, N] bf16."""
    return np.ascontiguousarray(a.reshape(4, 128, -1)).astype(BFNP)


def _colscal(a):
    """[512] bias -> [128, 4] per-partition fp32 column layout."""
    return np.ascontiguousarray(a.reshape(4, 128).T.astype(np.float32))


def _mprime_index():
    a, bb, f, i, j = np.meshgrid(
        np.arange(4), np.arange(4), np.arange(4), np.arange(4), np.arange(4),
        indexing="ij")
    return (f * 256 + (4 * a + i) * 16 + 4 * bb + j).reshape(-1)


def _fold_block(p, w):
    g, b = w[p + "_ln_g"], w[p + "_ln_b"]
    qkv1, wo1, bo1 = w[p + "_qkv1"], w[p + "_wo1"], w[p + "_bo1"]
    qkv2, wo2, bo2 = w[p + "_qkv2"], w[p + "_wo2"], w[p + "_bo2"]
    f1w, f1b = w[p + "_fc1w"], w[p + "_fc1b"]
    f2w, f2b = w[p + "_fc2w"], w[p + "_fc2b"]
    d = {}
    d["wq1"] = (g[0][:, None] * qkv1[0]) * SCALE
    d["bq1"] = (b[0] @ qkv1[0]) * SCALE
    d["wk1"] = g[0][:, None] * qkv1[1]          # k bias dropped (softmax-inv)
    d["wv1"] = g[0][:, None] * qkv1[2]
    bv1 = b[0] @ qkv1[2]
    d["wo1"] = wo1
    d["bo1"] = bo1 + bv1 @ wo1                  # v bias folded through softmax
    d["wq2"] = (g[1][:, None] * qkv2[0]) * SCALE
    d["bq2"] = (b[1] @ qkv2[0]) * SCALE
    d["wk2"] = qkv2[1]
    d["wv2"] = qkv2[2]
    d["wo2"], d["bo2"] = wo2, bo2
    d["wf1"] = g[2][:, None] * f1w
    d["bf1"] = b[2] @ f1w + f1b
    d["wf2"], d["bf2"] = f2w, f2b
    return d


def host_prep(inputs):
    """Full inputs -> list of 8 per-core input maps (shared weights aliased)."""
    w = {k: np.asarray(v) for k, v in inputs.items()}
    x = w["x"].astype(np.float32)
    actions = np.asarray(w["actions"])

    shared = {}
    shared["wp"] = np.ascontiguousarray(
        w["conv_w"].reshape(C, CIN * P * P).T).astype(BFNP)       # [48, 512]
    shared["bh"] = np.ascontiguousarray(
        w["head_b"].reshape(48, 1).astype(np.float32))

    for p in ("s", "g"):
        fb = _fold_block(p, w)
        wa = np.concatenate(
            [_w4(fb[n]) for n in ("wq1", "wk1", "wv1", "wo1", "wq2")], axis=2)
        wbl = [_w4(fb[n]) for n in ("wk2", "wv2", "wo2", "wf1", "wf2")]
        if p == "g":
            wbl.append(_w4(w["head_w"]))                           # [4,128,48]
        else:
            wbl.append(np.zeros((4, 128, 48), BFNP))
        wb = np.concatenate(wbl, axis=2)
        shared[p + "wa"] = np.ascontiguousarray(wa)
        shared[p + "wb"] = np.ascontiguousarray(wb)
        shared[p + "bias"] = np.ascontiguousarray(np.concatenate(
            [_colscal(fb[n]) for n in ("bq1", "bo1", "bq2", "bo2",
                                       "bf1", "bf2")], axis=1))    # [128, 24]

    # cross-attn selector matrices: R[p, ct, q] = rec[32*((2ct+(p>=64))%4), q]
    sel = np.zeros((4, 128, 128), np.float32)
    for ct in range(4):
        for pp in range(128):
            h = 2 * ct + (1 if pp >= 64 else 0)
            sel[ct, 32 * (h % 4), pp] = 1.0
    shared["sel"] = sel.astype(BFNP)
    same = (np.arange(128)[:, None] < 64) == (np.arange(128)[None, :] < 64)
    shared["wmask"] = np.ascontiguousarray(
        np.tile(same.astype(np.float32), (1, 8))).astype(BFNP)

    # positional embedding folded with conv bias: pe[c, (f, ph, pw)]
    pe_all = (w["conv_b"][None, None, :] + w["t_pos"][:, None, :]
              + w["sp_pos"][None, :, :])                           # [F, NP, C]

    per_core = []
    for c in range(8):
        b, t = c // 4, c % 4
        m = dict(shared)
        xs = x[b, :, 4 * t:4 * t + 4].reshape(CIN, 4, PD, P, PD, P)
        xs = xs.transpose(0, 3, 5, 1, 2, 4)        # (c, p, q, f, ph, pw)
        m["xpt"] = np.ascontiguousarray(
            xs.reshape(CIN * P * P, M)).astype(BFNP)
        pe = pe_all[4 * t:4 * t + 4].reshape(M, C).T               # [C, M]
        m["pe"] = np.ascontiguousarray(pe.reshape(4, 128, M)).astype(BFNP)
        ctx = (w["act_table"][actions[b]] + w["act_pos"][0])
        m["ctxt"] = np.ascontiguousarray(
            ctx.T.reshape(4, 128, S)).astype(BFNP)
        per_core.append(m)
    return per_core


# ---------------------------------------------------------------- device

class Dev:
    pass


class Pools:
    pass


def _declare_inputs(nc):
    d = Dev()
    d.nc = nc
    mkb = lambda name, shape: nc.dram_tensor(name, shape, BF16,
                                             kind="ExternalInput")
    mkf = lambda name, shape: nc.dram_tensor(name, shape, F32,
                                             kind="ExternalInput")
    d.xpt = mkb("xpt", [48, M])
    d.pe = mkb("pe", [4, 128, M])
    d.ctxt = mkb("ctxt", [4, 128, S])
    d.wp = mkb("wp", [48, C])
    d.bh = mkf("bh", [48, 1])
    d.sel = mkb("sel", [4, 128, 128])
    d.wmask = mkb("wmask", [128, 1024])
    for p in ("s", "g"):
        setattr(d, p + "wa", mkb(p + "wa", [4, 128, NWA]))
        setattr(d, p + "wb", mkb(p + "wb", [4, 128, NWB]))
        setattr(d, p + "bias", mkf(p + "bias", [128, 24]))
    d.out = nc.dram_tensor("out", [48, M], F32, kind="ExternalOutput")
    return d


def _dbg_out(d, p, t_sb):
    nc = d.nc
    outT = p.cpool.tile([48, M], F32, name="outT", tag="outT")
    nc.vector.tensor_copy(out=outT, in_=t_sb)
    nc.sync.dma_start(out=d.out.ap(), in_=outT)


def _load_whalf(d, p, dram, name):
    nc = d.nc
    n = dram.shape[2]
    t = p.wpool.tile([128, 4, n], BF16, name=name, tag="whalf")
    nc.sync.dma_start(out=t, in_=dram.ap().transpose([1, 0, 2]))
    return t


def _linear_fm(d, p, w_sb, col0, x_sb, out_sb, mode, bias_col=None):
    """Feature-major linear over tokens.  mode: 'act' (ACT identity+bias),
    'copy' (DVE copy, no bias), 'res' (h += ps + bias), 'gelu'."""
    nc = d.nc
    # ms outer: all ct tiles of token-half 0 finish first, so the next
    # LayerNorm's half-0 chain can overlap this linear's half-1 matmuls
    for ms in range(2):
        for ct in range(CT):
            ps = p.ps.tile([128, 512], F32, name="lin_ps", tag="ps")
            for kt in range(CT):
                nc.tensor.matmul(
                    ps, w_sb[:, kt, col0 + ct * 128:col0 + (ct + 1) * 128],
                    x_sb[:, kt, ms * 512:(ms + 1) * 512],
                    start=(kt == 0), stop=(kt == 3))
            sl = out_sb[:, ct, ms * 512:(ms + 1) * 512]
            if mode == "res":
                nc.vector.scalar_tensor_tensor(
                    out=sl, in0=ps, scalar=bias_col[:, ct:ct + 1], in1=sl,
                    op0=ALU.add, op1=ALU.add)
            elif mode == "copy":
                nc.scalar.copy(out=sl, in_=ps)
            elif mode == "act":
                nc.scalar.activation(out=sl, in_=ps, func=AF.Identity,
                                     bias=bias_col[:, ct:ct + 1], scale=1.0)
            elif mode == "gelu":
                nc.scalar.activation(out=sl, in_=ps, func=AF.Gelu,
                                     bias=bias_col[:, ct:ct + 1], scale=1.0)


def _v_proj(d, p, w_sb, col0, x_sb, v_sb):
    """Token-major value projection v_sb[128 tok, mt, C] (no bias)."""
    nc = d.nc
    for mt in range(8):
        ps = p.ps.tile([128, 512], F32, name="v_ps", tag="ps")
        for kt in range(CT):
            nc.tensor.matmul(
                ps, x_sb[:, kt, mt * 128:(mt + 1) * 128],
                w_sb[:, kt, col0:col0 + 512],
                start=(kt == 0), stop=(kt == 3))
        nc.scalar.copy(out=v_sb[:, mt, :], in_=ps)


def _ln(d, p, h_sb, z_sb):
    """z = (h - mean_c) * rsqrt(var_c + eps) via gpsimd partition allreduce.

    Processed per token-half so half 0's chain (DVE+POOL) overlaps both the
    preceding linear's half-1 matmuls and this LN's half-1 chain, and the
    following linear's half-0 matmuls can start before half 1 finishes."""
    nc = d.nc
    MH = M // 2
    for ms in range(2):
        msl = slice(ms * MH, (ms + 1) * MH)
        hh = h_sb[:, :, msl]
        sq = p.lnsq.tile([128, CT, MH], BF16, name="ln_sq", tag="lnsq")
        nc.scalar.activation(out=sq, in_=hh, func=AF.Square)
        c2 = p.lncomb.tile([128, 2, MH], BF16, name="ln_c2", tag="lncomb")
        nc.vector.tensor_add(c2, hh[:, 0:2, :], hh[:, 2:4, :])
        cs = p.lncs.tile([128, MH], BF16, name="ln_cs", tag="lncs")
        nc.vector.tensor_add(cs, c2[:, 0, :], c2[:, 1, :])
        d2 = p.lncomb.tile([128, 2, MH], BF16, name="ln_d2", tag="lncomb")
        nc.vector.tensor_add(d2, sq[:, 0:2, :], sq[:, 2:4, :])
        ds_ = p.lncs.tile([128, MH], BF16, name="ln_ds", tag="lncs")
        nc.vector.tensor_add(ds_, d2[:, 0, :], d2[:, 1, :])
        sh = p.lnar.tile([128, MH], BF16, name="ln_sh", tag="lnar")
        nc.gpsimd.partition_all_reduce(sh, cs, 128, ReduceOp.add)
        sqs = p.lnar.tile([128, MH], BF16, name="ln_sqs", tag="lnar")
        nc.gpsimd.partition_all_reduce(sqs, ds_, 128, ReduceOp.add)
        mu2 = p.lnt.tile([128, MH], BF16, name="ln_mu2", tag="lnt")
        nc.vector.tensor_mul(mu2, sh, sh)
        varu = p.lnt.tile([128, MH], BF16, name="ln_varu", tag="lnt")
        # varu = mu2/512 - sumsq  (= -512*var)
        nc.vector.scalar_tensor_tensor(
            out=varu, in0=mu2, scalar=1.0 / C,
            in1=sqs, op0=ALU.mult, op1=ALU.subtract)
        sd = p.lnt.tile([128, MH], BF16, name="ln_sd", tag="lnt")
        nc.scalar.activation(out=sd, in_=varu, func=AF.Sqrt,
                             bias=d.epscol, scale=-1.0 / C)
        r = p.lnt.tile([128, MH], BF16, name="ln_r", tag="lnt")
        with nc.allow_low_precision(reason="rel tolerance 2e-2"):
            nc.vector.reciprocal(r, sd)
        psi = p.lnt.tile([128, MH], BF16, name="ln_psi", tag="lnt")
        nc.vector.scalar_tensor_tensor(out=psi, in0=sh, scalar=-1.0 / C,
                                       in1=r, op0=ALU.mult, op1=ALU.mult)
        rb = r.unsqueeze(1).broadcast_to([128, CT, MH])
        pb = psi.unsqueeze(1).broadcast_to([128, CT, MH])
        zz = z_sb[:, :, msl]
        nc.vector.tensor_mul(zz, hh, rb)
        nc.vector.tensor_add(zz, zz, pb)


def _attn_s(d, p, qt, kt, v_sb, attn):
    """Block-s self attention: 4 windows (frames) x 256 tokens, 8 heads."""
    nc = d.nc
    for w in range(4):
        exs = []
        for g in range(2):
            scg = p.psA.tile([128, 4, 512], F32, name="scg_s", tag="psA")
            for hh in range(4):
                h_ = 4 * g + hh
                hp, ct = 64 * (h_ % 2), h_ // 2
                for st in range(2):
                    nc.tensor.matmul(
                        scg[:, hh, st * 256:(st + 1) * 256],
                        kt[hp:hp + 64, ct,
                           w * 256 + st * 128:w * 256 + st * 128 + 128],
                        qt[hp:hp + 64, ct, w * 256:(w + 1) * 256],
                        start=True, stop=True, tile_position=(hp, 0))
            ex = p.ex.tile([128, 4, 2, 256], BF16, name="ex_s", tag="ex")
            nc.scalar.activation(out=ex, in_=scg, func=AF.Exp)
            exs.append(ex)
        dn = p.psA.tile([1, 2048], F32, name="dn_s", tag="psA")
        for h_ in range(8):
            g, hh = h_ // 4, h_ % 4
            for st in range(2):
                nc.tensor.matmul(
                    dn[0:1, h_ * 256:(h_ + 1) * 256],
                    d.ones_col, exs[g][:, hh, st, :],
                    start=(st == 0), stop=(st == 1))
        rrow = p.rrow.tile([1, 2048], BF16, name="rr_s", tag="rrow")
        with nc.allow_low_precision(reason="rel tolerance 2e-2"):
            nc.vector.reciprocal(rrow, dn)
        rb = p.rb.tile([128, 2048], BF16, name="rb_s", tag="rb")
        nc.gpsimd.partition_broadcast(rb, rrow, 128)
        rbv = rb.rearrange("p (h q) -> p h q", h=8)
        for g in range(2):
            in1 = rbv[:, 4 * g:4 * g + 4, :].unsqueeze(2).broadcast_to(
                [128, 4, 2, 256])
            nc.vector.tensor_mul(exs[g], exs[g], in1)
        ob = p.psB.tile([128, 4, 256], F32, name="ob_s", tag="psB")
        for h_ in range(8):
            g, hh = h_ // 4, h_ % 4
            hp, ct = 64 * (h_ % 2), h_ // 2
            for st in range(2):
                nc.tensor.matmul(
                    ob[hp:hp + 64, ct, :],
                    v_sb[:, 2 * w + st, h_ * 64:h_ * 64 + 64],
                    exs[g][:, hh, st, :],
                    start=(st == 0), stop=(st == 1), tile_position=(0, hp))
        nc.scalar.copy(out=attn[:, :, w * 256:(w + 1) * 256], in_=ob)


def _attn_g(d, p, qt, kt, v_sb, attn):
    """Block-g self attention: 16 windows x 64 tokens processed as 8 window
    pairs; cross-window terms masked to exactly 0 after exp.

    Scores are banked by head parity (slice idx = 4*(h%2) + h//2) so all
    concurrent matmuls in a PSUM bank share one contraction row group."""
    nc = d.nc
    for wp in range(8):
        scg = p.psB.tile([128, 8, 128], F32, name="scg_g", tag="psB")
        for h_ in range(8):
            hp, ct = 64 * (h_ % 2), h_ // 2
            idx = 4 * (h_ % 2) + ct
            nc.tensor.matmul(
                scg[:, idx, :],
                kt[hp:hp + 64, ct, wp * 128:(wp + 1) * 128],
                qt[hp:hp + 64, ct, wp * 128:(wp + 1) * 128],
                start=True, stop=True, tile_position=(hp, 0))
        ex = p.ex.tile([128, 8, 128], BF16, name="ex_g", tag="ex")
        nc.scalar.activation(out=ex, in_=scg, func=AF.Exp)
        # zero the cross-window quadrants (exact block-diagonal attention)
        nc.vector.tensor_mul(
            ex, ex, d.wmask_sb.rearrange("p (h q) -> p h q", h=8))
        # psA is idle during block-g self-attention; using it for the
        # denominators lets the next window pair's scores start in psB
        # while this pair's normalization is still in flight.
        dn = p.psA.tile([1, 1024], F32, name="dn_g", tag="psA")
        for half in range(2):
            nc.tensor.matmul(
                dn[0:1, half * 512:(half + 1) * 512],
                d.ones_col, ex[:, 4 * half:4 * half + 4, :].rearrange(
                    "p a b -> p (a b)"),
                start=True, stop=True)
        rrow = p.rrow.tile([1, 1024], BF16, name="rr_g", tag="rrow")
        with nc.allow_low_precision(reason="rel tolerance 2e-2"):
            nc.vector.reciprocal(rrow, dn)
        rb = p.rb.tile([128, 1024], BF16, name="rb_g", tag="rb")
        nc.gpsimd.partition_broadcast(rb, rrow, 128)
        nc.vector.tensor_mul(ex, ex, rb.rearrange("p (h q) -> p h q", h=8))
        ob = p.ps.tile([128, 4, 128], F32, name="ob_g", tag="ps")
        for h_ in range(8):
            hp, ct = 64 * (h_ % 2), h_ // 2
            idx = 4 * (h_ % 2) + ct
            nc.tensor.matmul(
                ob[hp:hp + 64, ct, :],
                v_sb[:, wp, h_ * 64:h_ * 64 + 64],
                ex[:, idx, :],
                start=True, stop=True, tile_position=(0, hp))
        nc.scalar.copy(out=attn[:, :, wp * 128:(wp + 1) * 128], in_=ob)


def _cross_attn(d, p, prefix, wb_sb, qt2, attn):
    """Cross attention to the 16 action-context tokens."""
    nc = d.nc
    # ctx K feature-major [128, 4, 16]
    ktc = p.ktc.tile([128, CT, S], BF16, name=prefix + "ktc", tag="ktc")
    for ct in range(CT):
        ps = p.ps.tile([128, 512], F32, name="ktc_ps", tag="ps")
        for kt in range(CT):
            nc.tensor.matmul(
                ps[:, 0:S],
                wb_sb[:, kt, OB["wk2"] + ct * 128:OB["wk2"] + (ct + 1) * 128],
                d.ctxt_sb[:, kt, :], start=(kt == 0), stop=(kt == 3))
        nc.scalar.copy(out=ktc[:, ct, :], in_=ps[:, 0:S])
    # ctx V token-major [16, 512], replicated to the four 32-strips
    vc = p.vc.tile([128, C], BF16, name=prefix + "vc", tag="vc")
    ps = p.ps.tile([128, 512], F32, name="vc_ps", tag="ps")
    for kt in range(CT):
        nc.tensor.matmul(ps[0:S, :], d.ctxt_sb[:, kt, :],
                         wb_sb[:, kt, OB["wv2"]:OB["wv2"] + 512],
                         start=(kt == 0), stop=(kt == 3))
    nc.scalar.copy(out=vc[0:S, :], in_=ps[0:S, :])
    for q in range(1, 4):
        nc.sync.dma_start(out=vc[32 * q:32 * q + S, :], in_=vc[0:S, :])

    for ms in range(2):
        msl = slice(ms * 512, (ms + 1) * 512)
        sc = p.psB.tile([128, 2, 512], F32, name="sc_x", tag="psB")
        for h_ in range(8):
            g, hh = h_ // 4, h_ % 4
            hp, ct = 64 * (h_ % 2), h_ // 2
            nc.tensor.matmul(
                sc[32 * hh:32 * hh + S, g, :],
                ktc[hp:hp + 64, ct, :],
                qt2[hp:hp + 64, ct, msl],
                start=True, stop=True, tile_position=(hp, 32 * hh))
        exc = p.ex.tile([128, 2, 512], BF16, name="ex_x", tag="ex")
        nc.scalar.activation(out=exc, in_=sc, func=AF.Exp)
        dn = p.psB.tile([128, 2, 512], F32, name="dn_x", tag="psB")
        nc.vector.memset(dn, 1.0)
        for h_ in range(8):
            g, hh = h_ // 4, h_ % 4
            nc.tensor.matmul(
                dn[32 * hh:32 * hh + 1, g, :],
                d.ones_col[32 * hh:32 * hh + S, 0:1],
                exc[32 * hh:32 * hh + S, g, :],
                start=True, stop=True, skip_group_check=True,
                tile_position=(32 * hh, 32 * hh))
        rec = p.rec.tile([128, 2, 512], BF16, name="rec_x", tag="rec")
        with nc.allow_low_precision(reason="rel tolerance 2e-2"):
            nc.vector.reciprocal(rec, dn)
        R = p.psA.tile([128, 4, 512], F32, name="R_x", tag="psA")
        for ct in range(CT):
            nc.tensor.matmul(R[:, ct, :], d.sel_sb[:, ct, :],
                             rec[:, ct // 2, :], start=True, stop=True)
        rsb = p.rsb.tile([128, 4, 512], BF16, name="rsb_x", tag="rsb")
        nc.scalar.copy(out=rsb, in_=R)
        obp = p.psA.tile([128, 4, 512], F32, name="ob_x", tag="psA")
        for h_ in range(8):
            g, hh = h_ // 4, h_ % 4
            hp, ct = 64 * (h_ % 2), h_ // 2
            nc.tensor.matmul(
                obp[hp:hp + 64, ct, :],
                vc[32 * hh:32 * hh + S, h_ * 64:h_ * 64 + 64],
                exc[32 * hh:32 * hh + S, g, :],
                start=True, stop=True, tile_position=(32 * hh, hp))
        nc.vector.tensor_mul(attn[:, :, msl], obp, rsb)


def _block(d, p, prefix, h_sb, dbg_base):
    nc = d.nc
    wa = _load_whalf(d, p, getattr(d, prefix + "wa"), prefix + "wa")
    z = p.state.tile([128, CT, M], BF16, name="z", tag="zbuf")
    attn = p.state.tile([128, CT, M], BF16, name="attn", tag="attnbuf")
    qt = p.state.tile([128, CT, M], BF16, name="qt", tag="qbuf")
    kt = p.state.tile([128, CT, M], BF16, name="kt", tag="kbuf")
    bias = getattr(d, prefix + "bias_sb")

    _ln(d, p, h_sb, z)
    if _STAGE == dbg_base + 0:
        return _dbg_out(d, p, z[0:48, 0, :])
    _linear_fm(d, p, wa, OA["wq1"], z, qt, "act", bias[:, 0:4])
    _linear_fm(d, p, wa, OA["wk1"], z, kt, "copy")
    v_sb = p.state.tile([128, 8, C], BF16, name="v", tag="vbuf")
    _v_proj(d, p, wa, OA["wv1"], z, v_sb)
    if _STAGE == dbg_base + 1:
        return _dbg_out(d, p, qt[0:48, 0, :])
    if prefix == "s":
        _attn_s(d, p, qt, kt, v_sb, attn)
    else:
        _attn_g(d, p, qt, kt, v_sb, attn)
    if _STAGE == dbg_base + 2:
        return _dbg_out(d, p, attn[0:48, 0, :])
    _linear_fm(d, p, wa, OA["wo1"], attn, h_sb, "res", bias[:, 4:8])

    if _STAGE == dbg_base + 3:
        return _dbg_out(d, p, h_sb[0:48, 0, :])
    _ln(d, p, h_sb, z)
    qt2 = p.state.tile([128, CT, M], BF16, name="qt2", tag="kbuf")
    _linear_fm(d, p, wa, OA["wq2"], z, qt2, "act", bias[:, 8:12])
    wb = _load_whalf(d, p, getattr(d, prefix + "wb"), prefix + "wb")
    _cross_attn(d, p, prefix, wb, qt2, attn)
    if _STAGE == dbg_base + 4:
        return _dbg_out(d, p, attn[0:48, 0, :])
    _linear_fm(d, p, wb, OB["wo2"], attn, h_sb, "res", bias[:, 12:16])
    if _STAGE == dbg_base + 5:
        return _dbg_out(d, p, h_sb[0:48, 0, :])

    _ln(d, p, h_sb, z)
    gbuf = p.state.tile([128, CT, M], BF16, name="gbuf", tag="scratch")
    _linear_fm(d, p, wb, OB["wf1"], z, gbuf, "gelu", bias[:, 16:20])
    _linear_fm(d, p, wb, OB["wf2"], gbuf, h_sb, "res", bias[:, 20:24])
    return wb


def _body(d, p):
    nc = d.nc

    xpt = p.cpool.tile([48, M], BF16, name="xpt_sb", tag="xpt")
    nc.sync.dma_start(out=xpt, in_=d.xpt.ap())
    wp = p.cpool.tile([48, C], BF16, name="wp_sb", tag="wp")
    nc.sync.dma_start(out=wp, in_=d.wp.ap())
    pe = p.cpool.tile([128, CT, M], BF16, name="pe_sb", tag="pe")
    nc.sync.dma_start(out=pe, in_=d.pe.ap().transpose([1, 0, 2]))
    d.ctxt_sb = p.cpool.tile([128, CT, S], BF16, name="ctxt_sb", tag="ctxt")
    nc.sync.dma_start(out=d.ctxt_sb, in_=d.ctxt.ap().transpose([1, 0, 2]))
    d.sel_sb = p.cpool.tile([128, CT, 128], BF16, name="sel_sb", tag="sel")
    nc.sync.dma_start(out=d.sel_sb, in_=d.sel.ap().transpose([1, 0, 2]))
    d.wmask_sb = p.cpool.tile([128, 1024], BF16, name="wmask_sb",
                              tag="wmask")
    nc.sync.dma_start(out=d.wmask_sb, in_=d.wmask.ap())
    d.bh_sb = p.cpool.tile([48, 1], F32, name="bh_sb", tag="bh")
    nc.sync.dma_start(out=d.bh_sb, in_=d.bh.ap())
    for pref in ("s", "g"):
        t = p.cpool.tile([128, 24], F32, name=pref + "bias_sb",
                         tag=pref + "bias")
        nc.sync.dma_start(out=t, in_=getattr(d, pref + "bias").ap())
        setattr(d, pref + "bias_sb", t)

    ones_col = p.cpool.tile([128, 1], BF16, name="ones_col", tag="ones")
    nc.vector.memset(ones_col, 1.0)
    d.ones_col = ones_col
    epscol = p.cpool.tile([128, 1], F32, name="epscol", tag="eps")
    nc.vector.memset(epscol, EPS)
    d.epscol = epscol

    h = p.state.tile([128, CT, M], BF16, name="h", tag="hbuf")
    # patch embedding + (conv bias + positional) add
    for ct in range(CT):
        for ms in range(2):
            msl = slice(ms * 512, (ms + 1) * 512)
            ps = p.ps.tile([128, 512], F32, name="pe_ps", tag="ps")
            nc.tensor.matmul(ps, wp[:, ct * 128:(ct + 1) * 128],
                             xpt[:, msl], start=True, stop=True)
            nc.vector.tensor_add(h[:, ct, msl], ps, pe[:, ct, msl])
    if _STAGE <= 1:
        return _dbg_out(d, p, h[0:48, 0, :])

    _block(d, p, "s", h, 20)
    if _STAGE <= 2 or (20 <= _STAGE <= 25):
        if _STAGE <= 2:
            return _dbg_out(d, p, h[0:48, 0, :])
        return

    # permute tokens to m' (window-major for the 4x4x4 windows)
    hg = p.state.tile([128, CT, M], BF16, name="hg", tag="zbuf2")
    for ct in range(CT):
        for a in range(4):
            src = h[:, ct, :].rearrange("p (f ph pw) -> p f ph pw",
                                        f=4, ph=16)[:, :, 4 * a:4 * a + 4, :]
            src = src.rearrange("p f i (b j) -> p b f i j", b=4)
            nc.vector.tensor_copy(
                out=hg[:, ct, a * 256:(a + 1) * 256].rearrange(
                    "p (b f i j) -> p b f i j", b=4, f=4, i=4),
                in_=src)
    if _STAGE <= 3:
        return _dbg_out(d, p, hg[0:48, 0, :])

    gwb = _block(d, p, "g", hg, 30)
    if _STAGE <= 4 or (30 <= _STAGE <= 35):
        if _STAGE <= 4:
            return _dbg_out(d, p, hg[0:48, 0, :])
        return

    # output head (m' order; host undoes the permutation)
    outT = p.cpool.tile([48, M], F32, name="outT", tag="outT")
    for ms in range(2):
        msl = slice(ms * 512, (ms + 1) * 512)
        ps = p.ps.tile([128, 512], F32, name="hd_ps", tag="ps")
        for kt in range(CT):
            nc.tensor.matmul(ps[0:48, :],
                             gwb[:, kt, OB["wh"]:OB["wh"] + 48],
                             hg[:, kt, msl], start=(kt == 0), stop=(kt == 3))
        nc.scalar.activation(out=outT[:, msl], in_=ps[0:48, :],
                             func=AF.Identity, bias=d.bh_sb[:, 0:1],
                             scale=1.0)
    nc.sync.dma_start(out=d.out.ap(), in_=outT)


def _build_nc(loop_n=1):
    key = loop_n
    if key in _BUILD_CACHE:
        return _BUILD_CACHE[key]
    nc = bacc.Bacc("TRN2", target_bir_lowering=False, debug=False,
                   num_devices=8)
    d = _declare_inputs(nc)
    with tile.TileContext(nc) as tc:
        p = Pools()
        import contextlib
        stack = contextlib.ExitStack()
        with stack:
            p.ps = stack.enter_context(
                tc.tile_pool(name="ps", bufs=2, space="PSUM"))
            p.psA = stack.enter_context(
                tc.tile_pool(name="psA", bufs=1, space="PSUM"))
            p.psB = stack.enter_context(
                tc.tile_pool(name="psB", bufs=1, space="PSUM"))
            p.wpool = stack.enter_context(tc.tile_pool(name="wpool", bufs=2))
            p.cpool = stack.enter_context(tc.tile_pool(name="cpool", bufs=1))
            p.state = stack.enter_context(tc.tile_pool(name="state", bufs=1))
            p.lnsq = stack.enter_context(tc.tile_pool(name="lnsq", bufs=2))
            p.lncomb = stack.enter_context(
                tc.tile_pool(name="lncomb", bufs=3))
            p.lncs = stack.enter_context(tc.tile_pool(name="lncs", bufs=4))
            p.lnar = stack.enter_context(tc.tile_pool(name="lnar", bufs=4))
            p.lnt = stack.enter_context(tc.tile_pool(name="lnt", bufs=6))
            p.ex = stack.enter_context(tc.tile_pool(name="ex", bufs=2))
            p.rb = stack.enter_context(tc.tile_pool(name="rb", bufs=2))
            p.rrow = stack.enter_context(tc.tile_pool(name="rrow", bufs=2))
            p.rec = stack.enter_context(tc.tile_pool(name="rec", bufs=2))
            p.rsb = stack.enter_context(tc.tile_pool(name="rsb", bufs=2))
            p.ktc = stack.enter_context(tc.tile_pool(name="ktc", bufs=1))
            p.vc = stack.enter_context(tc.tile_pool(name="vc", bufs=1))
            for _ in range(loop_n):
                _body(d, p)
    nc.compile()
    _BUILD_CACHE[key] = nc
    return nc


# ---------------------------------------------------------------- runner

_MPRIME = None
_RUNNERS = {}


class _Runner:
    """AOT-compiled executable + device-resident inputs for one nc."""

    def __init__(self, nc, n_cores=8):
        import jax
        from jax.sharding import Mesh, PartitionSpec
        try:
            from jax.experimental.shard_map import shard_map
        except ImportError:
            from jax import shard_map
        import concourse.bass2jax as b2j
        self.jax = jax
        b2j.install_neuronx_cc_hook()
        self.nc = nc
        self.n_cores = n_cores
        partition_name = (nc.partition_id_tensor.name
                          if nc.partition_id_tensor else None)
        in_names, out_names, out_avals, zero_outs = [], [], [], []
        for alloc in nc.m.functions[0].allocations:
            if not isinstance(alloc, mybir.MemoryLocationSet):
                continue
            name = alloc.memorylocations[0].name
            if alloc.kind == "ExternalInput":
                if name != partition_name:
                    in_names.append(name)
            elif alloc.kind == "ExternalOutput":
                out_names.append(name)
                shape = tuple(alloc.tensor_shape)
                dtype = mybir.dt.np(alloc.dtype)
                out_avals.append(jax.core.ShapedArray(shape, dtype))
                zero_outs.append(np.zeros(shape, dtype))
        self.in_names, self.out_names = in_names, out_names
        self.out_avals, self.zero_outs = out_avals, zero_outs
        n_params, n_outs = len(in_names), len(out_avals)
        all_in = list(in_names) + list(out_names)
        if partition_name is not None:
            all_in.append(partition_name)
        donate = tuple(range(n_params, n_params + n_outs))

        def _fn(*args):
            operands = list(args)
            if partition_name is not None:
                operands.append(b2j.partition_id_tensor())
            return tuple(b2j._bass_exec_p.bind(
                *operands, out_avals=tuple(out_avals),
                in_names=tuple(all_in), out_names=tuple(out_names),
                lowering_input_output_aliases=(),
                sim_require_finite=False, sim_require_nnan=False, nc=nc))

        devices = jax.devices()[:n_cores]
        self.mesh = Mesh(np.asarray(devices), ("core",))
        in_specs = (PartitionSpec("core"),) * (n_params + n_outs)
        out_specs = (PartitionSpec("core"),) * len(out_names)
        jitted = jax.jit(
            shard_map(_fn, mesh=self.mesh, in_specs=in_specs,
                      out_specs=out_specs, check_rep=False),
            donate_argnums=donate, keep_unused=True)
        arg_structs = []
        self._in_shapes = {}
        dummy = None
        self.compiled = None
        self._jitted = jitted
        self.sharding = jax.sharding.NamedSharding(
            self.mesh, PartitionSpec("core"))
        self.dev_inputs = None

    def _concat(self, in_maps):
        return [np.concatenate([np.asarray(m[n]) for m in in_maps], axis=0)
                for n in self.in_names]

    def _compile(self, concat_in):
        jax = self.jax
        zeros = [np.zeros((self.n_cores * z.shape[0], *z.shape[1:]), z.dtype)
                 for z in self.zero_outs]
        structs = [jax.ShapeDtypeStruct(a.shape, a.dtype)
                   for a in concat_in + zeros]
        self.compiled = self._jitted.lower(*structs).compile()
        self._zero_shapes = [(z.shape, z.dtype) for z in zeros]

    def put_inputs(self, in_maps):
        jax = self.jax
        concat_in = self._concat(in_maps)
        if self.compiled is None:
            self._compile(concat_in)
        self.dev_inputs = [jax.device_put(a, self.sharding)
                           for a in concat_in]
        jax.block_until_ready(self.dev_inputs)

    def _fresh_zeros(self):
        jax = self.jax
        zs = [jax.device_put(np.zeros(s, d), self.sharding)
              for s, d in self._zero_shapes]
        jax.block_until_ready(zs)
        return zs

    def execute(self, fetch=True):
        jax = self.jax
        outs = self.compiled(*self.dev_inputs, *self._fresh_zeros())
        jax.block_until_ready(outs)
        if not fetch:
            return None
        res = []
        for c in range(self.n_cores):
            m = {}
            for i, name in enumerate(self.out_names):
                full = np.asarray(outs[i])
                m[name] = full.reshape(self.n_cores,
                                       *self.out_avals[i].shape)[c]
            res.append(m)
        return res

    def execute_timed(self):
        """One timed execution: zeros staged outside, only dispatch+exec."""
        import time
        jax = self.jax
        zs = self._fresh_zeros()
        t0 = time.perf_counter()
        outs = self.compiled(*self.dev_inputs, *zs)
        jax.block_until_ready(outs)
        t1 = time.perf_counter()
        for o in outs:
            o.delete()
        return t1 - t0


def get_runner(loop_n=1):
    if loop_n not in _RUNNERS:
        _RUNNERS[loop_n] = _Runner(_build_nc(loop_n))
    return _RUNNERS[loop_n]


def kernel(**inputs) -> np.ndarray:
    global _MPRIME
    if _MPRIME is None:
        _MPRIME = _mprime_index()
    in_maps = host_prep(inputs)
    try:
        r = get_runner(1)
        r.put_inputs(in_maps)
        results = r.execute(fetch=True)
    except Exception:
        # fall back to the stock SPMD path if the AOT runner misbehaves
        res = run_bass_kernel_spmd(_build_nc(1), in_maps,
                                   core_ids=list(range(8)))
        results = res.results
    out = np.zeros((B, CIN, F, IMG, IMG), np.float32)
    for c in range(8):
        b, t = c // 4, c % 4
        ot = results[c]["out"]                     # [48, M] m'-order
        om = np.empty_like(ot)
        om[:, _MPRIME] = ot
        om = om.reshape(P, P, CIN, 4, PD, PD)
        om = om.transpose(2, 3, 4, 0, 5, 1)
        out[b, :, 4 * t:4 * t + 4] = om.reshape(CIN, 4, IMG, IMG)
    return out


if __name__ == "__main__":
    import reference
    import jax
    cpu = jax.devices("cpu")[0]
    with jax.default_device(cpu):
        ins = reference.setup_inputs()
        ins = {k: np.asarray(v) for k, v in ins.items()}
        exp = np.asarray(reference.reference(**ins))
    act = kernel(**ins)
    err = np.abs(act - exp).max() / (np.abs(exp).max() + 1e-12)
    print("Relative error:", err)


# revision 10
# speedup vs baseline: 1.2189x; 1.1270x over previous
# Trainium2 Bass kernel v2 for nn_DiT_89086211653924 (windowed-attention video
# DiT).  Same sharding as v1: core c -> (batch c//4, temporal group c%4), 1024
# tokens per core, zero collectives.
#
# v2 changes vs v1:
#  - all matmuls in bf16 (4x PE rate), activations stored bf16, psum fp32
#  - LayerNorm stats via gpsimd partition_all_reduce (13 ops vs ~30)
#  - attention softmax batched per window group: one exp per 4 heads (multi-
#    bank PSUM AP), denominators via ones-matmuls, reciprocal rows broadcast
#    via gpsimd partition_broadcast
#  - block-g (4x4x4 window) scores computed per window-PAIR as one
#    [128k, 128q] matmul per head; cross-window quadrants of exp(scores)
#    are zeroed by a 0/1 mask (exact block-diagonal attention, half the
#    score/ob matmuls)
#  - K-projection bias dropped (softmax-invariant), V-projection bias folded
#    into the out-projection bias on the host
#  - weights shipped as 4 packed tensors (2 per block), few big DMAs
#  - AOT-cached jit executable + on-device input caching so repeated calls
#    do not pay client-side recompile/transfer
import sys

sys.path.insert(0, "/opt/trn_rl_repo")

import numpy as np
import ml_dtypes

import concourse.bacc as bacc
import concourse.mybir as mybir
import concourse.tile as tile
import concourse.bass as bass
from concourse.bass_utils import run_bass_kernel_spmd

F32 = mybir.dt.float32
BF16 = mybir.dt.bfloat16
AF = mybir.ActivationFunctionType
ALU = mybir.AluOpType
ReduceOp = bass.bass_isa.ReduceOp
BFNP = ml_dtypes.bfloat16

B, CIN, F, IMG, P = 2, 3, 16, 64, 4
PD = IMG // P          # 16 patches per side
NP = PD * PD           # 256 patches per frame
C, NH = 512, 8
HD = C // NH           # 64
S = 16                 # action context length
SCALE = float(1.0 / np.sqrt(HD))
M = 1024               # tokens per core (4 frames)
CT = 4                 # c tiles of 128
EPS = 1e-5
PEN = 5.0              # sqrt of window penalty (exactly representable in bf16)

# weight column offsets inside each half-pack (units of C=512 columns)
# half A: wq1 wk1 wv1 wo1 wq2   half B: wk2 wv2 wo2 wf1 wf2 [wh]
OA = {"wq1": 0, "wk1": 512, "wv1": 1024, "wo1": 1536, "wq2": 2048}
OB = {"wk2": 0, "wv2": 512, "wo2": 1024, "wf1": 1536, "wf2": 2048, "wh": 2560}
NWA, NWB = 2560, 2608

_BUILD_CACHE = {}

import os as _os
_STAGE = int(_os.environ.get("DIT_STAGE", "99"))


# ---------------------------------------------------------------- host prep

def _w4(a):
    """[512, N] k-major weight -> [4, 128, N] bf16."""
    return np.ascontiguousarray(a.reshape(4, 128, -1)).astype(BFNP)


def _colscal(a):
    """[512] bias -> [128, 4] per-partition fp32 column layout."""
    return np.ascontiguousarray(a.reshape(4, 128).T.astype(np.float32))


def _mprime_index():
    a, bb, f, i, j = np.meshgrid(
        np.arange(4), np.arange(4), np.arange(4), np.arange(4), np.arange(4),
        indexing="ij")
    return (f * 256 + (4 * a + i) * 16 + 4 * bb + j).reshape(-1)


def _fold_block(p, w):
    g, b = w[p + "_ln_g"], w[p + "_ln_b"]
    qkv1, wo1, bo1 = w[p + "_qkv1"], w[p + "_wo1"], w[p + "_bo1"]
    qkv2, wo2, bo2 = w[p + "_qkv2"], w[p + "_wo2"], w[p + "_bo2"]
    f1w, f1b = w[p + "_fc1w"], w[p + "_fc1b"]
    f2w, f2b = w[p + "_fc2w"], w[p + "_fc2b"]
    d = {}
    d["wq1"] = (g[0][:, None] * qkv1[0]) * SCALE
    d["bq1"] = (b[0] @ qkv1[0]) * SCALE
    d["wk1"] = g[0][:, None] * qkv1[1]          # k bias dropped (softmax-inv)
    d["wv1"] = g[0][:, None] * qkv1[2]
    bv1 = b[0] @ qkv1[2]
    d["wo1"] = wo1
    d["bo1"] = bo1 + bv1 @ wo1                  # v bias folded through softmax
    d["wq2"] = (g[1][:, None] * qkv2[0]) * SCALE
    d["bq2"] = (b[1] @ qkv2[0]) * SCALE
    d["wk2"] = qkv2[1]
    d["wv2"] = qkv2[2]
    d["wo2"], d["bo2"] = wo2, bo2
    d["wf1"] = g[2][:, None] * f1w
    d["bf1"] = b[2] @ f1w + f1b
    d["wf2"], d["bf2"] = f2w, f2b
    return d


def host_prep(inputs):
    """Full inputs -> list of 8 per-core input maps (shared weights aliased)."""
    w = {k: np.asarray(v) for k, v in inputs.items()}
    x = w["x"].astype(np.float32)
    actions = np.asarray(w["actions"])

    shared = {}
    shared["wp"] = np.ascontiguousarray(
        w["conv_w"].reshape(C, CIN * P * P).T).astype(BFNP)       # [48, 512]
    shared["bh"] = np.ascontiguousarray(
        w["head_b"].reshape(48, 1).astype(np.float32))

    for p in ("s", "g"):
        fb = _fold_block(p, w)
        wa = np.concatenate(
            [_w4(fb[n]) for n in ("wq1", "wk1", "wv1", "wo1", "wq2")], axis=2)
        wbl = [_w4(fb[n]) for n in ("wk2", "wv2", "wo2", "wf1", "wf2")]
        if p == "g":
            wbl.append(_w4(w["head_w"]))                           # [4,128,48]
        else:
            wbl.append(np.zeros((4, 128, 48), BFNP))
        wb = np.concatenate(wbl, axis=2)
        shared[p + "wa"] = np.ascontiguousarray(wa)
        shared[p + "wb"] = np.ascontiguousarray(wb)
        shared[p + "bias"] = np.ascontiguousarray(np.concatenate(
            [_colscal(fb[n]) for n in ("bq1", "bo1", "bq2", "bo2",
                                       "bf1", "bf2")], axis=1))    # [128, 24]

    # cross-attn selector matrices: R[p, ct, q] = rec[32*((2ct+(p>=64))%4), q]
    sel = np.zeros((4, 128, 128), np.float32)
    for ct in range(4):
        for pp in range(128):
            h = 2 * ct + (1 if pp >= 64 else 0)
            sel[ct, 32 * (h % 4), pp] = 1.0
    shared["sel"] = sel.astype(BFNP)
    same = (np.arange(128)[:, None] < 64) == (np.arange(128)[None, :] < 64)
    shared["wmask"] = np.ascontiguousarray(
        np.tile(same.astype(np.float32), (1, 8))).astype(BFNP)

    # positional embedding folded with conv bias: pe[c, (f, ph, pw)]
    pe_all = (w["conv_b"][None, None, :] + w["t_pos"][:, None, :]
              + w["sp_pos"][None, :, :])                           # [F, NP, C]

    per_core = []
    for c in range(8):
        b, t = c // 4, c % 4
        m = dict(shared)
        xs = x[b, :, 4 * t:4 * t + 4].reshape(CIN, 4, PD, P, PD, P)
        xs = xs.transpose(0, 3, 5, 1, 2, 4)        # (c, p, q, f, ph, pw)
        m["xpt"] = np.ascontiguousarray(
            xs.reshape(CIN * P * P, M)).astype(BFNP)
        pe = pe_all[4 * t:4 * t + 4].reshape(M, C).T               # [C, M]
        m["pe"] = np.ascontiguousarray(pe.reshape(4, 128, M)).astype(BFNP)
        ctx = (w["act_table"][actions[b]] + w["act_pos"][0])
        m["ctxt"] = np.ascontiguousarray(
            ctx.T.reshape(4, 128, S)).astype(BFNP)
        per_core.append(m)
    return per_core


# ---------------------------------------------------------------- device

class Dev:
    pass


class Pools:
    pass


def _declare_inputs(nc):
    d = Dev()
    d.nc = nc
    mkb = lambda name, shape: nc.dram_tensor(name, shape, BF16,
                                             kind="ExternalInput")
    mkf = lambda name, shape: nc.dram_tensor(name, shape, F32,
                                             kind="ExternalInput")
    d.xpt = mkb("xpt", [48, M])
    d.pe = mkb("pe", [4, 128, M])
    d.ctxt = mkb("ctxt", [4, 128, S])
    d.wp = mkb("wp", [48, C])
    d.bh = mkf("bh", [48, 1])
    d.sel = mkb("sel", [4, 128, 128])
    d.wmask = mkb("wmask", [128, 1024])
    for p in ("s", "g"):
        setattr(d, p + "wa", mkb(p + "wa", [4, 128, NWA]))
        setattr(d, p + "wb", mkb(p + "wb", [4, 128, NWB]))
        setattr(d, p + "bias", mkf(p + "bias", [128, 24]))
    d.out = nc.dram_tensor("out", [48, M], F32, kind="ExternalOutput")
    return d


def _dbg_out(d, p, t_sb):
    nc = d.nc
    outT = p.cpool.tile([48, M], F32, name="outT", tag="outT")
    nc.vector.tensor_copy(out=outT, in_=t_sb)
    nc.sync.dma_start(out=d.out.ap(), in_=outT)


def _load_whalf(d, p, dram, name):
    nc = d.nc
    n = dram.shape[2]
    t = p.wpool.tile([128, 4, n], BF16, name=name, tag="whalf")
    nc.sync.dma_start(out=t, in_=dram.ap().transpose([1, 0, 2]))
    return t


def _linear_fm(d, p, w_sb, col0, x_sb, out_sb, mode, bias_col=None):
    """Feature-major linear over tokens.  mode: 'act' (ACT identity+bias),
    'copy' (DVE copy, no bias), 'res' (h += ps + bias), 'gelu'."""
    nc = d.nc
    # ms outer: all ct tiles of token-half 0 finish first, so the next
    # LayerNorm's half-0 chain can overlap this linear's half-1 matmuls
    for ms in range(2):
        for ct in range(CT):
            ps = p.ps.tile([128, 512], F32, name="lin_ps", tag="ps")
            for kt in range(CT):
                nc.tensor.matmul(
                    ps, w_sb[:, kt, col0 + ct * 128:col0 + (ct + 1) * 128],
                    x_sb[:, kt, ms * 512:(ms + 1) * 512],
                    start=(kt == 0), stop=(kt == 3))
            sl = out_sb[:, ct, ms * 512:(ms + 1) * 512]
            if mode == "res":
                nc.vector.scalar_tensor_tensor(
                    out=sl, in0=ps, scalar=bias_col[:, ct:ct + 1], in1=sl,
                    op0=ALU.add, op1=ALU.add)
            elif mode == "copy":
                nc.vector.tensor_copy(out=sl, in_=ps)
            elif mode == "act":
                nc.scalar.activation(out=sl, in_=ps, func=AF.Identity,
                                     bias=bias_col[:, ct:ct + 1], scale=1.0)
            elif mode == "gelu":
                nc.scalar.activation(out=sl, in_=ps, func=AF.Gelu,
                                     bias=bias_col[:, ct:ct + 1], scale=1.0)


def _v_proj(d, p, w_sb, col0, x_sb, v_sb):
    """Token-major value projection v_sb[128 tok, mt, C] (no bias)."""
    nc = d.nc
    for mt in range(8):
        ps = p.ps.tile([128, 512], F32, name="v_ps", tag="ps")
        for kt in range(CT):
            nc.tensor.matmul(
                ps, x_sb[:, kt, mt * 128:(mt + 1) * 128],
                w_sb[:, kt, col0:col0 + 512],
                start=(kt == 0), stop=(kt == 3))
        nc.scalar.copy(out=v_sb[:, mt, :], in_=ps)


def _ln(d, p, h_sb, z_sb):
    """z = (h - mean_c) * rsqrt(var_c + eps) via gpsimd partition allreduce.

    Processed per token-half so half 0's chain (DVE+POOL) overlaps both the
    preceding linear's half-1 matmuls and this LN's half-1 chain, and the
    following linear's half-0 matmuls can start before half 1 finishes."""
    nc = d.nc
    MH = M // 2
    for ms in range(2):
        msl = slice(ms * MH, (ms + 1) * MH)
        hh = h_sb[:, :, msl]
        sq = p.lnsq.tile([128, CT, MH], BF16, name="ln_sq", tag="lnsq")
        nc.scalar.activation(out=sq, in_=hh, func=AF.Square)
        c2 = p.lncomb.tile([128, 2, MH], BF16, name="ln_c2", tag="lncomb")
        nc.vector.tensor_add(c2, hh[:, 0:2, :], hh[:, 2:4, :])
        cs = p.lncs.tile([128, MH], BF16, name="ln_cs", tag="lncs")
        nc.vector.tensor_add(cs, c2[:, 0, :], c2[:, 1, :])
        d2 = p.lncomb.tile([128, 2, MH], BF16, name="ln_d2", tag="lncomb")
        nc.vector.tensor_add(d2, sq[:, 0:2, :], sq[:, 2:4, :])
        ds_ = p.lncs.tile([128, MH], BF16, name="ln_ds", tag="lncs")
        nc.vector.tensor_add(ds_, d2[:, 0, :], d2[:, 1, :])
        sh = p.lnar.tile([128, MH], BF16, name="ln_sh", tag="lnar")
        nc.gpsimd.partition_all_reduce(sh, cs, 128, ReduceOp.add)
        sqs = p.lnar.tile([128, MH], BF16, name="ln_sqs", tag="lnar")
        nc.gpsimd.partition_all_reduce(sqs, ds_, 128, ReduceOp.add)
        mu2 = p.lnt.tile([128, MH], BF16, name="ln_mu2", tag="lnt")
        nc.vector.tensor_mul(mu2, sh, sh)
        varu = p.lnt.tile([128, MH], BF16, name="ln_varu", tag="lnt")
        # varu = mu2/512 - sumsq  (= -512*var)
        nc.vector.scalar_tensor_tensor(
            out=varu, in0=mu2, scalar=1.0 / C,
            in1=sqs, op0=ALU.mult, op1=ALU.subtract)
        sd = p.lnt.tile([128, MH], BF16, name="ln_sd", tag="lnt")
        nc.scalar.activation(out=sd, in_=varu, func=AF.Sqrt,
                             bias=d.epscol, scale=-1.0 / C)
        r = p.lnt.tile([128, MH], BF16, name="ln_r", tag="lnt")
        with nc.allow_low_precision(reason="rel tolerance 2e-2"):
            nc.vector.reciprocal(r, sd)
        psi = p.lnt.tile([128, MH], BF16, name="ln_psi", tag="lnt")
        nc.vector.scalar_tensor_tensor(out=psi, in0=sh, scalar=-1.0 / C,
                                       in1=r, op0=ALU.mult, op1=ALU.mult)
        rb = r.unsqueeze(1).broadcast_to([128, CT, MH])
        pb = psi.unsqueeze(1).broadcast_to([128, CT, MH])
        zz = z_sb[:, :, msl]
        nc.vector.tensor_mul(zz, hh, rb)
        nc.vector.tensor_add(zz, zz, pb)


def _attn_s(d, p, qt, kt, v_sb, attn):
    """Block-s self attention: 4 windows (frames) x 256 tokens, 8 heads."""
    nc = d.nc
    for w in range(4):
        exs = []
        for g in range(2):
            scg = p.psA.tile([128, 4, 512], F32, name="scg_s", tag="psA")
            for hh in range(4):
                h_ = 4 * g + hh
                hp, ct = 64 * (h_ % 2), h_ // 2
                for st in range(2):
                    nc.tensor.matmul(
                        scg[:, hh, st * 256:(st + 1) * 256],
                        kt[hp:hp + 64, ct,
                           w * 256 + st * 128:w * 256 + st * 128 + 128],
                        qt[hp:hp + 64, ct, w * 256:(w + 1) * 256],
                        start=True, stop=True, tile_position=(hp, 0))
            ex = p.ex.tile([128, 4, 2, 256], BF16, name="ex_s", tag="ex")
            nc.scalar.activation(out=ex, in_=scg, func=AF.Exp)
            exs.append(ex)
        dn = p.psA.tile([1, 2048], F32, name="dn_s", tag="psA")
        for h_ in range(8):
            g, hh = h_ // 4, h_ % 4
            for st in range(2):
                nc.tensor.matmul(
                    dn[0:1, h_ * 256:(h_ + 1) * 256],
                    d.ones_col, exs[g][:, hh, st, :],
                    start=(st == 0), stop=(st == 1))
        rrow = p.rrow.tile([1, 2048], BF16, name="rr_s", tag="rrow")
        with nc.allow_low_precision(reason="rel tolerance 2e-2"):
            nc.vector.reciprocal(rrow, dn)
        rb = p.rb.tile([128, 2048], BF16, name="rb_s", tag="rb")
        nc.gpsimd.partition_broadcast(rb, rrow, 128)
        rbv = rb.rearrange("p (h q) -> p h q", h=8)
        for g in range(2):
            in1 = rbv[:, 4 * g:4 * g + 4, :].unsqueeze(2).broadcast_to(
                [128, 4, 2, 256])
            nc.vector.tensor_mul(exs[g], exs[g], in1)
        ob = p.psB.tile([128, 4, 256], F32, name="ob_s", tag="psB")
        for h_ in range(8):
            g, hh = h_ // 4, h_ % 4
            hp, ct = 64 * (h_ % 2), h_ // 2
            for st in range(2):
                nc.tensor.matmul(
                    ob[hp:hp + 64, ct, :],
                    v_sb[:, 2 * w + st, h_ * 64:h_ * 64 + 64],
                    exs[g][:, hh, st, :],
                    start=(st == 0), stop=(st == 1), tile_position=(0, hp))
        nc.scalar.copy(out=attn[:, :, w * 256:(w + 1) * 256], in_=ob)


def _attn_g(d, p, qt, kt, v_sb, attn):
    """Block-g self attention: 16 windows x 64 tokens processed as 8 window
    pairs; cross-window terms masked to exactly 0 after exp.

    Scores are banked by head parity (slice idx = 4*(h%2) + h//2) so all
    concurrent matmuls in a PSUM bank share one contraction row group."""
    nc = d.nc
    for wp in range(8):
        scg = p.psB.tile([128, 8, 128], F32, name="scg_g", tag="psB")
        for h_ in range(8):
            hp, ct = 64 * (h_ % 2), h_ // 2
            idx = 4 * (h_ % 2) + ct
            nc.tensor.matmul(
                scg[:, idx, :],
                kt[hp:hp + 64, ct, wp * 128:(wp + 1) * 128],
                qt[hp:hp + 64, ct, wp * 128:(wp + 1) * 128],
                start=True, stop=True, tile_position=(hp, 0))
        ex = p.ex.tile([128, 8, 128], BF16, name="ex_g", tag="ex")
        nc.scalar.activation(out=ex, in_=scg, func=AF.Exp)
        # zero the cross-window quadrants (exact block-diagonal attention)
        nc.vector.tensor_mul(
            ex, ex, d.wmask_sb.rearrange("p (h q) -> p h q", h=8))
        # psA is idle during block-g self-attention; using it for the
        # denominators lets the next window pair's scores start in psB
        # while this pair's normalization is still in flight.
        dn = p.psA.tile([1, 1024], F32, name="dn_g", tag="psA")
        for half in range(2):
            nc.tensor.matmul(
                dn[0:1, half * 512:(half + 1) * 512],
                d.ones_col, ex[:, 4 * half:4 * half + 4, :].rearrange(
                    "p a b -> p (a b)"),
                start=True, stop=True)
        rrow = p.rrow.tile([1, 1024], BF16, name="rr_g", tag="rrow")
        with nc.allow_low_precision(reason="rel tolerance 2e-2"):
            nc.vector.reciprocal(rrow, dn)
        rb = p.rb.tile([128, 1024], BF16, name="rb_g", tag="rb")
        nc.gpsimd.partition_broadcast(rb, rrow, 128)
        nc.vector.tensor_mul(ex, ex, rb.rearrange("p (h q) -> p h q", h=8))
        ob = p.ps.tile([128, 4, 128], F32, name="ob_g", tag="ps")
        for h_ in range(8):
            hp, ct = 64 * (h_ % 2), h_ // 2
            idx = 4 * (h_ % 2) + ct
            nc.tensor.matmul(
                ob[hp:hp + 64, ct, :],
                v_sb[:, wp, h_ * 64:h_ * 64 + 64],
                ex[:, idx, :],
                start=True, stop=True, tile_position=(0, hp))
        nc.scalar.copy(out=attn[:, :, wp * 128:(wp + 1) * 128], in_=ob)


def _cross_attn(d, p, prefix, wb_sb, qt2, attn):
    """Cross attention to the 16 action-context tokens."""
    nc = d.nc
    # ctx K feature-major [128, 4, 16]
    ktc = p.ktc.tile([128, CT, S], BF16, name=prefix + "ktc", tag="ktc")
    for ct in range(CT):
        ps = p.ps.tile([128, 512], F32, name="ktc_ps", tag="ps")
        for kt in range(CT):
            nc.tensor.matmul(
                ps[:, 0:S],
                wb_sb[:, kt, OB["wk2"] + ct * 128:OB["wk2"] + (ct + 1) * 128],
                d.ctxt_sb[:, kt, :], start=(kt == 0), stop=(kt == 3))
        nc.scalar.copy(out=ktc[:, ct, :], in_=ps[:, 0:S])
    # ctx V token-major [16, 512], replicated to the four 32-strips
    vc = p.vc.tile([128, C], BF16, name=prefix + "vc", tag="vc")
    ps = p.ps.tile([128, 512], F32, name="vc_ps", tag="ps")
    for kt in range(CT):
        nc.tensor.matmul(ps[0:S, :], d.ctxt_sb[:, kt, :],
                         wb_sb[:, kt, OB["wv2"]:OB["wv2"] + 512],
                         start=(kt == 0), stop=(kt == 3))
    nc.scalar.copy(out=vc[0:S, :], in_=ps[0:S, :])
    for q in range(1, 4):
        nc.sync.dma_start(out=vc[32 * q:32 * q + S, :], in_=vc[0:S, :])

    for ms in range(2):
        msl = slice(ms * 512, (ms + 1) * 512)
        sc = p.psB.tile([128, 2, 512], F32, name="sc_x", tag="psB")
        for h_ in range(8):
            g, hh = h_ // 4, h_ % 4
            hp, ct = 64 * (h_ % 2), h_ // 2
            nc.tensor.matmul(
                sc[32 * hh:32 * hh + S, g, :],
                ktc[hp:hp + 64, ct, :],
                qt2[hp:hp + 64, ct, msl],
                start=True, stop=True, tile_position=(hp, 32 * hh))
        exc = p.ex.tile([128, 2, 512], BF16, name="ex_x", tag="ex")
        nc.scalar.activation(out=exc, in_=sc, func=AF.Exp)
        dn = p.psB.tile([128, 2, 512], F32, name="dn_x", tag="psB")
        nc.vector.memset(dn, 1.0)
        for h_ in range(8):
            g, hh = h_ // 4, h_ % 4
            nc.tensor.matmul(
                dn[32 * hh:32 * hh + 1, g, :],
                d.ones_col[32 * hh:32 * hh + S, 0:1],
                exc[32 * hh:32 * hh + S, g, :],
                start=True, stop=True, skip_group_check=True,
                tile_position=(32 * hh, 32 * hh))
        rec = p.rec.tile([128, 2, 512], BF16, name="rec_x", tag="rec")
        with nc.allow_low_precision(reason="rel tolerance 2e-2"):
            nc.vector.reciprocal(rec, dn)
        R = p.psA.tile([128, 4, 512], F32, name="R_x", tag="psA")
        for ct in range(CT):
            nc.tensor.matmul(R[:, ct, :], d.sel_sb[:, ct, :],
                             rec[:, ct // 2, :], start=True, stop=True)
        rsb = p.rsb.tile([128, 4, 512], BF16, name="rsb_x", tag="rsb")
        nc.vector.tensor_copy(out=rsb, in_=R)
        obp = p.psA.tile([128, 4, 512], F32, name="ob_x", tag="psA")
        for h_ in range(8):
            g, hh = h_ // 4, h_ % 4
            hp, ct = 64 * (h_ % 2), h_ // 2
            nc.tensor.matmul(
                obp[hp:hp + 64, ct, :],
                vc[32 * hh:32 * hh + S, h_ * 64:h_ * 64 + 64],
                exc[32 * hh:32 * hh + S, g, :],
                start=True, stop=True, tile_position=(32 * hh, hp))
        nc.vector.tensor_mul(attn[:, :, msl], obp, rsb)


def _block(d, p, prefix, h_sb, dbg_base):
    nc = d.nc
    wa = _load_whalf(d, p, getattr(d, prefix + "wa"), prefix + "wa")
    z = p.state.tile([128, CT, M], BF16, name="z", tag="zbuf")
    attn = p.state.tile([128, CT, M], BF16, name="attn", tag="attnbuf")
    qt = p.state.tile([128, CT, M], BF16, name="qt", tag="qbuf")
    kt = p.state.tile([128, CT, M], BF16, name="kt", tag="kbuf")
    bias = getattr(d, prefix + "bias_sb")

    _ln(d, p, h_sb, z)
    if _STAGE == dbg_base + 0:
        return _dbg_out(d, p, z[0:48, 0, :])
    _linear_fm(d, p, wa, OA["wq1"], z, qt, "act", bias[:, 0:4])
    _linear_fm(d, p, wa, OA["wk1"], z, kt, "copy")
    v_sb = p.state.tile([128, 8, C], BF16, name="v", tag="vbuf")
    _v_proj(d, p, wa, OA["wv1"], z, v_sb)
    if _STAGE == dbg_base + 1:
        return _dbg_out(d, p, qt[0:48, 0, :])
    if prefix == "s":
        _attn_s(d, p, qt, kt, v_sb, attn)
    else:
        _attn_g(d, p, qt, kt, v_sb, attn)
    if _STAGE == dbg_base + 2:
        return _dbg_out(d, p, attn[0:48, 0, :])
    _linear_fm(d, p, wa, OA["wo1"], attn, h_sb, "res", bias[:, 4:8])

    if _STAGE == dbg_base + 3:
        return _dbg_out(d, p, h_sb[0:48, 0, :])
    _ln(d, p, h_sb, z)
    qt2 = p.state.tile([128, CT, M], BF16, name="qt2", tag="kbuf")
    _linear_fm(d, p, wa, OA["wq2"], z, qt2, "act", bias[:, 8:12])
    wb = _load_whalf(d, p, getattr(d, prefix + "wb"), prefix + "wb")
    _cross_attn(d, p, prefix, wb, qt2, attn)
    if _STAGE == dbg_base + 4:
        return _dbg_out(d, p, attn[0:48, 0, :])
    _linear_fm(d, p, wb, OB["wo2"], attn, h_sb, "res", bias[:, 12:16])
    if _STAGE == dbg_base + 5:
        return _dbg_out(d, p, h_sb[0:48, 0, :])

    _ln(d, p, h_sb, z)
    gbuf = p.state.tile([128, CT, M], BF16, name="gbuf", tag="scratch")
    _linear_fm(d, p, wb, OB["wf1"], z, gbuf, "gelu", bias[:, 16:20])
    _linear_fm(d, p, wb, OB["wf2"], gbuf, h_sb, "res", bias[:, 20:24])
    return wb


def _body(d, p):
    nc = d.nc

    xpt = p.cpool.tile([48, M], BF16, name="xpt_sb", tag="xpt")
    nc.sync.dma_start(out=xpt, in_=d.xpt.ap())
    wp = p.cpool.tile([48, C], BF16, name="wp_sb", tag="wp")
    nc.sync.dma_start(out=wp, in_=d.wp.ap())
    pe = p.cpool.tile([128, CT, M], BF16, name="pe_sb", tag="pe")
    nc.sync.dma_start(out=pe, in_=d.pe.ap().transpose([1, 0, 2]))
    d.ctxt_sb = p.cpool.tile([128, CT, S], BF16, name="ctxt_sb", tag="ctxt")
    nc.sync.dma_start(out=d.ctxt_sb, in_=d.ctxt.ap().transpose([1, 0, 2]))
    d.sel_sb = p.cpool.tile([128, CT, 128], BF16, name="sel_sb", tag="sel")
    nc.sync.dma_start(out=d.sel_sb, in_=d.sel.ap().transpose([1, 0, 2]))
    d.wmask_sb = p.cpool.tile([128, 1024], BF16, name="wmask_sb",
                              tag="wmask")
    nc.sync.dma_start(out=d.wmask_sb, in_=d.wmask.ap())
    d.bh_sb = p.cpool.tile([48, 1], F32, name="bh_sb", tag="bh")
    nc.sync.dma_start(out=d.bh_sb, in_=d.bh.ap())
    for pref in ("s", "g"):
        t = p.cpool.tile([128, 24], F32, name=pref + "bias_sb",
                         tag=pref + "bias")
        nc.sync.dma_start(out=t, in_=getattr(d, pref + "bias").ap())
        setattr(d, pref + "bias_sb", t)

    ones_col = p.cpool.tile([128, 1], BF16, name="ones_col", tag="ones")
    nc.vector.memset(ones_col, 1.0)
    d.ones_col = ones_col
    epscol = p.cpool.tile([128, 1], F32, name="epscol", tag="eps")
    nc.vector.memset(epscol, EPS)
    d.epscol = epscol

    h = p.state.tile([128, CT, M], BF16, name="h", tag="hbuf")
    # patch embedding + (conv bias + positional) add
    for ct in range(CT):
        for ms in range(2):
            msl = slice(ms * 512, (ms + 1) * 512)
            ps = p.ps.tile([128, 512], F32, name="pe_ps", tag="ps")
            nc.tensor.matmul(ps, wp[:, ct * 128:(ct + 1) * 128],
                             xpt[:, msl], start=True, stop=True)
            nc.vector.tensor_add(h[:, ct, msl], ps, pe[:, ct, msl])
    if _STAGE <= 1:
        return _dbg_out(d, p, h[0:48, 0, :])

    _block(d, p, "s", h, 20)
    if _STAGE <= 2 or (20 <= _STAGE <= 25):
        if _STAGE <= 2:
            return _dbg_out(d, p, h[0:48, 0, :])
        return

    # permute tokens to m' (window-major for the 4x4x4 windows)
    hg = p.state.tile([128, CT, M], BF16, name="hg", tag="zbuf2")
    for a in range(4):
        for ct in range(CT):
            src = h[:, ct, :].rearrange("p (f ph pw) -> p f ph pw",
                                        f=4, ph=16)[:, :, 4 * a:4 * a + 4, :]
            src = src.rearrange("p f i (b j) -> p b f i j", b=4)
            nc.vector.tensor_copy(
                out=hg[:, ct, a * 256:(a + 1) * 256].rearrange(
                    "p (b f i j) -> p b f i j", b=4, f=4, i=4),
                in_=src)
    if _STAGE <= 3:
        return _dbg_out(d, p, hg[0:48, 0, :])

    gwb = _block(d, p, "g", hg, 30)
    if _STAGE <= 4 or (30 <= _STAGE <= 35):
        if _STAGE <= 4:
            return _dbg_out(d, p, hg[0:48, 0, :])
        return

    # output head (m' order; host undoes the permutation)
    outT = p.cpool.tile([48, M], F32, name="outT", tag="outT")
    for ms in range(2):
        msl = slice(ms * 512, (ms + 1) * 512)
        ps = p.ps.tile([128, 512], F32, name="hd_ps", tag="ps")
        for kt in range(CT):
            nc.tensor.matmul(ps[0:48, :],
                             gwb[:, kt, OB["wh"]:OB["wh"] + 48],
                             hg[:, kt, msl], start=(kt == 0), stop=(kt == 3))
        nc.scalar.activation(out=outT[:, msl], in_=ps[0:48, :],
                             func=AF.Identity, bias=d.bh_sb[:, 0:1],
                             scale=1.0)
    nc.sync.dma_start(out=d.out.ap(), in_=outT)


def _build_nc(loop_n=1):
    key = loop_n
    if key in _BUILD_CACHE:
        return _BUILD_CACHE[key]
    nc = bacc.Bacc("TRN2", target_bir_lowering=False, debug=False,
                   num_devices=8)
    d = _declare_inputs(nc)
    with tile.TileContext(nc) as tc:
        p = Pools()
        import contextlib
        stack = contextlib.ExitStack()
        with stack:
            p.ps = stack.enter_context(
                tc.tile_pool(name="ps", bufs=2, space="PSUM"))
            p.psA = stack.enter_context(
                tc.tile_pool(name="psA", bufs=1, space="PSUM"))
            p.psB = stack.enter_context(
                tc.tile_pool(name="psB", bufs=1, space="PSUM"))
            p.wpool = stack.enter_context(tc.tile_pool(name="wpool", bufs=2))
            p.cpool = stack.enter_context(tc.tile_pool(name="cpool", bufs=1))
            p.state = stack.enter_context(tc.tile_pool(name="state", bufs=1))
            p.lnsq = stack.enter_context(tc.tile_pool(name="lnsq", bufs=2))
            p.lncomb = stack.enter_context(
                tc.tile_pool(name="lncomb", bufs=3))
            p.lncs = stack.enter_context(tc.tile_pool(name="lncs", bufs=4))
            p.lnar = stack.enter_context(tc.tile_pool(name="lnar", bufs=4))
            p.lnt = stack.enter_context(tc.tile_pool(name="lnt", bufs=6))
            p.ex = stack.enter_context(tc.tile_pool(name="ex", bufs=2))
            p.rb = stack.enter_context(tc.tile_pool(name="rb", bufs=2))
            p.rrow = stack.enter_context(tc.tile_pool(name="rrow", bufs=2))
            p.rec = stack.enter_context(tc.tile_pool(name="rec", bufs=2))
            p.rsb = stack.enter_context(tc.tile_pool(name="rsb", bufs=2))
            p.ktc = stack.enter_context(tc.tile_pool(name="ktc", bufs=1))
            p.vc = stack.enter_context(tc.tile_pool(name="vc", bufs=1))
            for _ in range(loop_n):
                _body(d, p)
    nc.compile()
    _BUILD_CACHE[key] = nc
    return nc


# ---------------------------------------------------------------- runner

_MPRIME = None
_RUNNERS = {}


class _Runner:
    """AOT-compiled executable + device-resident inputs for one nc."""

    def __init__(self, nc, n_cores=8):
        import jax
        from jax.sharding import Mesh, PartitionSpec
        try:
            from jax.experimental.shard_map import shard_map
        except ImportError:
            from jax import shard_map
        import concourse.bass2jax as b2j
        self.jax = jax
        b2j.install_neuronx_cc_hook()
        self.nc = nc
        self.n_cores = n_cores
        partition_name = (nc.partition_id_tensor.name
                          if nc.partition_id_tensor else None)
        in_names, out_names, out_avals, zero_outs = [], [], [], []
        for alloc in nc.m.functions[0].allocations:
            if not isinstance(alloc, mybir.MemoryLocationSet):
                continue
            name = alloc.memorylocations[0].name
            if alloc.kind == "ExternalInput":
                if name != partition_name:
                    in_names.append(name)
            elif alloc.kind == "ExternalOutput":
                out_names.append(name)
                shape = tuple(alloc.tensor_shape)
                dtype = mybir.dt.np(alloc.dtype)
                out_avals.append(jax.core.ShapedArray(shape, dtype))
                zero_outs.append(np.zeros(shape, dtype))
        self.in_names, self.out_names = in_names, out_names
        self.out_avals, self.zero_outs = out_avals, zero_outs
        n_params, n_outs = len(in_names), len(out_avals)
        all_in = list(in_names) + list(out_names)
        if partition_name is not None:
            all_in.append(partition_name)
        donate = tuple(range(n_params, n_params + n_outs))

        def _fn(*args):
            operands = list(args)
            if partition_name is not None:
                operands.append(b2j.partition_id_tensor())
            return tuple(b2j._bass_exec_p.bind(
                *operands, out_avals=tuple(out_avals),
                in_names=tuple(all_in), out_names=tuple(out_names),
                lowering_input_output_aliases=(),
                sim_require_finite=False, sim_require_nnan=False, nc=nc))

        devices = jax.devices()[:n_cores]
        self.mesh = Mesh(np.asarray(devices), ("core",))
        in_specs = (PartitionSpec("core"),) * (n_params + n_outs)
        out_specs = (PartitionSpec("core"),) * len(out_names)
        jitted = jax.jit(
            shard_map(_fn, mesh=self.mesh, in_specs=in_specs,
                      out_specs=out_specs, check_rep=False),
            donate_argnums=donate, keep_unused=True)
        arg_structs = []
        self._in_shapes = {}
        dummy = None
        self.compiled = None
        self._jitted = jitted
        self.sharding = jax.sharding.NamedSharding(
            self.mesh, PartitionSpec("core"))
        self.dev_inputs = None

    def _concat(self, in_maps):
        return [np.concatenate([np.asarray(m[n]) for m in in_maps], axis=0)
                for n in self.in_names]

    def _compile(self, concat_in):
        jax = self.jax
        zeros = [np.zeros((self.n_cores * z.shape[0], *z.shape[1:]), z.dtype)
                 for z in self.zero_outs]
        structs = [jax.ShapeDtypeStruct(a.shape, a.dtype)
                   for a in concat_in + zeros]
        self.compiled = self._jitted.lower(*structs).compile()
        self._zero_shapes = [(z.shape, z.dtype) for z in zeros]

    def put_inputs(self, in_maps):
        jax = self.jax
        concat_in = self._concat(in_maps)
        if self.compiled is None:
            self._compile(concat_in)
        self.dev_inputs = [jax.device_put(a, self.sharding)
                           for a in concat_in]
        jax.block_until_ready(self.dev_inputs)

    def _fresh_zeros(self):
        jax = self.jax
        zs = [jax.device_put(np.zeros(s, d), self.sharding)
              for s, d in self._zero_shapes]
        jax.block_until_ready(zs)
        return zs

    def execute(self, fetch=True):
        jax = self.jax
        outs = self.compiled(*self.dev_inputs, *self._fresh_zeros())
        jax.block_until_ready(outs)
        if not fetch:
            return None
        res = []
        for c in range(self.n_cores):
            m = {}
            for i, name in enumerate(self.out_names):
                full = np.asarray(outs[i])
                m[name] = full.reshape(self.n_cores,
                                       *self.out_avals[i].shape)[c]
            res.append(m)
        return res

    def execute_timed(self):
        """One timed execution: zeros staged outside, only dispatch+exec."""
        import time
        jax = self.jax
        zs = self._fresh_zeros()
        t0 = time.perf_counter()
        outs = self.compiled(*self.dev_inputs, *zs)
        jax.block_until_ready(outs)
        t1 = time.perf_counter()
        for o in outs:
            o.delete()
        return t1 - t0


def get_runner(loop_n=1):
    if loop_n not in _RUNNERS:
        _RUNNERS[loop_n] = _Runner(_build_nc(loop_n))
    return _RUNNERS[loop_n]


def kernel(**inputs) -> np.ndarray:
    global _MPRIME
    if _MPRIME is None:
        _MPRIME = _mprime_index()
    in_maps = host_prep(inputs)
    try:
        r = get_runner(1)
        r.put_inputs(in_maps)
        results = r.execute(fetch=True)
    except Exception:
        # fall back to the stock SPMD path if the AOT runner misbehaves
        res = run_bass_kernel_spmd(_build_nc(1), in_maps,
                                   core_ids=list(range(8)))
        results = res.results
    out = np.zeros((B, CIN, F, IMG, IMG), np.float32)
    for c in range(8):
        b, t = c // 4, c % 4
        ot = results[c]["out"]                     # [48, M] m'-order
        om = np.empty_like(ot)
        om[:, _MPRIME] = ot
        om = om.reshape(P, P, CIN, 4, PD, PD)
        om = om.transpose(2, 3, 4, 0, 5, 1)
        out[b, :, 4 * t:4 * t + 4] = om.reshape(CIN, 4, IMG, IMG)
    return out


if __name__ == "__main__":
    import reference
    import jax
    cpu = jax.devices("cpu")[0]
    with jax.default_device(cpu):
        ins = reference.setup_inputs()
        ins = {k: np.asarray(v) for k, v in ins.items()}
        exp = np.asarray(reference.reference(**ins))
    act = kernel(**ins)
    err = np.abs(act - exp).max() / (np.abs(exp).max() + 1e-12)
    print("Relative error:", err)
